# revision 40
# baseline (speedup 1.0000x reference)
"""Trainium2 Bass kernel for nn_CPCA (CPC-action loss).

Strategy: data-parallel over the env dim n (64 envs/core on 8 cores).
v4 (from the v3 indirect-gather baseline, 319us -> ~239us):
  - negatives are gathered AND transposed on the host into a chunk-major
    fp8 tensor streamed to SBUF by plain DMA.  This removes v3's 120
    serialized indirect DMAs (~1.1us each), 240 PE transposes and 120
    PSUM->SBUF copies that rate-limited the first 120us.
  - DMA issue order prioritized: GRU-critical tensors first (scan starts
    ~10us in), negatives/gin/weights stream in consumption order.
  - all PSUM tiles are single-bank [128,512] with a 6-deep rotation;
    the finer recycle granularity removes most PE psum-wait stalls.
  - GRU step emission is split by state-half dependency: kt01+aut
    partial-accumulations (needing only the previous step's first
    state half) issue before the kt23 closes, so the PE bridges the
    serial chain tail; r-gates and n-gates go first, z-gates ride in
    the chain's shadow.  The state stays in fp8 (tC8 tiles) and the
    update chain runs per-ct-tile to halve each link's latency.
  - bhh_n rides the n-matmul bias row so the r-mult is a plain wide
    tensor_tensor; gi_n (+bih_n) is host-precomputed (gin).
  - L1 matmuls for 21 of the 32 negative chunks run during the scan as
    PE filler ("parking": pre-activation copied to SBUF fp8, the
    relu(x+AT) applied post-scan) keeping the PE pstate ramped.
  - sections (AT/pos/neg pipelines) drain post-scan, software-pipelined
    L1(m)|L2(m-1)|L3(m-2); only each fi's AT stage runs at its step
    (its n8 state tile is recycled two steps later).
  - logit rows go PSUM -> small SBUF row -> DRAM, bounced back per
    8/4/4-chunk piece into a 128-row layout; softplus accumulation runs
    in two stages with exp/ln grouped so the act table loads exactly
    once per function family, overlapping the drain.
Per-core partial sums (pos_loss_sum, neg_loss_sum) are combined with
the host-side mask count into the scalar loss.
"""
import sys

if '/opt/trn_rl_repo' not in sys.path:
    sys.path.insert(0, '/opt/trn_rl_repo')

import numpy as np
import ml_dtypes

BF16 = ml_dtypes.bfloat16
FP8 = ml_dtypes.float8_e4m3   # IEEE e4m3 (max 240) == TRN fp8_exp4

N, T, H, TS, FS, K, A, ED, NNEG = 512, 128, 512, 6, 2, 8, 17, 32, 20
NCORE = 8
NE = N // NCORE          # 64 envs per core
P = NE * TS              # 384 positions per core (per unroll index)
PF = FS * P              # 768
NSLOT = FS * P * NNEG    # 15360 negative slots per core
SC = 24 * NNEG           # 480 slots (24 positions) per matmul sub-chunk
NSC = NSLOT // SC        # 32 sub-chunks (16 per unroll index)
SCF = NSC // FS          # 16
HKC = H // 128           # 4
NH = NSLOT // FS         # 7680 neg logits per fi
PH = P                   # 384 pos logits per fi

_PROG_CACHE = {}


# ----------------------------------------------------------------------------
# custom DVE op: out = relu(in0 + in1)   (in1 may be a stride-0 broadcast)
# ----------------------------------------------------------------------------

def _relu_add_op():
    from concourse import dve_ops
    from concourse.dve_spec import Spec, Src0, Src1, relu, lower
    from concourse.dve_uop import DveOpSpec

    name = "RELU_ADD_CPCA"
    for op in dve_ops.OPS:
        if op.name == name:
            return op

    def _ref(in0, in1, c0, c1, c2):
        x = np.asarray(in0, np.float32) + \
            np.asarray(in1, np.float32).reshape(np.asarray(in0).shape)
        return np.maximum(
            np.nan_to_num(x, nan=0.0, posinf=np.inf, neginf=-np.inf), 0)

    spec = Spec(body=relu(Src0 + Src1), reference=_ref)
    shas = {}
    for ver in ("v3", "v4"):
        tmp = DveOpSpec(name=name, opcode=31, uops=lower(spec, ver=ver),
                        rd1_en=True)
        shas[ver] = tmp.sha(ver)
    op = dve_ops.DveOp(name, spec, subdim=False, uops_sha=shas)
    dve_ops.OPS.append(op)
    dve_ops.CUSTOM_DVE_SPECS[name] = spec
    dve_ops._SUB_OPCODE_FOR_NAME[name] = (
        dve_ops._CUSTOM_DVE_ROW_BASE + len(dve_ops.OPS) - 1)
    assert dve_ops._SUB_OPCODE_FOR_NAME[name] < 0x20
    return op


# ----------------------------------------------------------------------------
# host-side input preparation (sharding / layout / index metadata only)
# ----------------------------------------------------------------------------

def _prep(inputs):
    acts = np.asarray(inputs['actions']).astype(np.int64)          # [N,T,1]
    nd = np.asarray(inputs['not_dones'], np.float32)               # [N,T,1]
    vld = np.asarray(inputs['valids']).astype(bool)                # [N,T,1]
    ri = np.asarray(inputs['rnn_inputs'], np.float32)              # [N,T,H]
    ro = np.asarray(inputs['rnn_outputs'], np.float32)             # [N,T,H]
    embw = np.asarray(inputs['embed_w'], np.float32)               # [A,ED]
    wih = np.asarray(inputs['gru_w_ih'], np.float32)               # [3H,ED]
    whh = np.asarray(inputs['gru_w_hh'], np.float32)               # [3H,H]
    bih = np.asarray(inputs['gru_b_ih'], np.float32)               # [3H]
    bhh = np.asarray(inputs['gru_b_hh'], np.float32)               # [3H]
    w1 = np.asarray(inputs['p_w1'], np.float32)                    # [H,2H]
    b1 = np.asarray(inputs['p_b1'], np.float32)                    # [H]
    w2 = np.asarray(inputs['p_w2'], np.float32)                    # [H,H]
    b2 = np.asarray(inputs['p_b2'], np.float32)                    # [H]
    w3 = np.asarray(inputs['p_w3'], np.float32)                    # [1,H]
    b3 = np.asarray(inputs['p_b3'], np.float32)                    # [1]
    tsub = np.asarray(inputs['time_subsample']).astype(np.int64)   # [TS]
    usub = np.asarray(inputs['unroll_subsample']).astype(np.int64) # [FS]
    negi = np.asarray(inputs['neg_indices']).astype(np.int64)      # [FS*TS*N*NNEG]
    maxk = int(np.asarray(inputs['max_k']))
    assert maxk == K, maxk
    assert tsub.shape == (TS,) and usub.shape == (FS,)

    forder = np.argsort(usub, kind='stable')                       # consumption order

    # ---- shared (replicated) tensors -------------------------------------
    def dr_std(w):
        # [p, g, i, m] = w[m, (2g+i)*128 + p]
        return np.ascontiguousarray(
            w.T.reshape(2, 2, 128, -1).transpose(2, 0, 1, 3)).astype(FP8)

    # GRU combined lhsT: k-tiles 0-3 = whh.T chunks, 4 = wih.T (+bias row 32),
    # 5 = zeros.  Gates r/z get bih+bhh via the bias row; the n-gate columns
    # of the aut plane carry ONLY bhh_n in the bias row (wih_n zeroed - the
    # ih part arrives host-precomputed via gin), so the n psum is directly
    # gh_n + bhh_n and the r-mult is a plain tensor_tensor multiply.
    wk = np.zeros((6, 128, 3 * H), np.float32)
    wk[:4] = whh.T.reshape(4, 128, 3 * H)
    wk[4, :ED, :2 * H] = wih.T[:, :2 * H]
    wk[4, ED] = np.concatenate([(bih + bhh)[:2 * H], bhh[2 * H:]])
    whhc = np.ascontiguousarray(
        wk.reshape(3, 2, 128, 3 * H).transpose(2, 0, 1, 3)).astype(FP8)

    w1ap = dr_std(w1[:, :H])
    w1bp = dr_std(w1[:, H:])
    w2p = dr_std(w2)
    # layer-3 weights broadcast to all 128 output rows — skinny (M<4) DR
    # ldweights fail the walrus ISA check; only PSUM partition 0 is read.
    w3p = np.ascontiguousarray(np.broadcast_to(
        w3[0].reshape(2, 2, 128).transpose(2, 0, 1)[..., None],
        (128, 2, 2, 128))).astype(FP8)

    b1_dev = np.ascontiguousarray(b1.reshape(HKC, 128).T)
    b2_dev = np.ascontiguousarray(b2.reshape(HKC, 128).T)
    b3c = np.broadcast_to(np.array([b3[0], -b3[0]], np.float32), (128, 2)).copy()

    # negatives pool in fp8 (same values the v3 device path consumed)
    pool8 = ri.reshape(N * T, H).astype(FP8)

    # ---- per-core views ---------------------------------------------------
    ks = np.arange(K)
    tq = tsub[None, :] + ks[:, None]                                # [K,TS]
    ok_au = tq <= T - 2
    a_idx = acts[:, np.clip(tq, 0, T - 1), 0]                       # [N,K,TS]
    au_full = embw[a_idx] * ok_au[None, :, :, None]                 # [N,K,TS,ED]

    tf = tsub[None, :] + usub[:, None]                              # [FS,TS]
    ok_ft = tf <= T - 2
    ft_full = np.where(ok_ft[None, :, :, None],
                       ri[:, np.clip(tf + 1, 0, T - 1)], 0.0)       # [N,FS,TS,H]

    vm = ((nd[:, :, 0] > 0) & vld[:, :, 0]).astype(np.float32)      # [N,T]
    vmk = np.where(ok_au[None], vm[:, np.clip(tq, 0, T - 1)], 0.0)  # [N,K,TS]
    cum = np.cumprod(vmk, axis=1)                                   # [N,K,TS]
    maskf = cum[:, usub, :]                                         # [N,FS,TS]

    negi4 = negi.reshape(FS, N, TS, NNEG)

    in_maps = []
    denoms = []
    for c in range(NCORE):
        sl = slice(c * NE, (c + 1) * NE)

        # h0: [128, 4, P] dev[p,kc,j] = ro[i, ts_s, kc*128+p], j = i*TS+s
        h0 = ro[sl][:, tsub].reshape(P, H).T                        # [H,P]
        ht0 = np.ascontiguousarray(h0.reshape(HKC, 128, P).transpose(1, 0, 2))
        ht08 = ht0.astype(FP8)

        # aut2: [128, K, 2, P]: plane 0 = action embedding rows 0-31 +
        # constant-1 bias row 32; plane 1 = zeros (DoubleRow zero k-tile)
        au_c = au_full[sl].transpose(1, 0, 2, 3).reshape(K, P, ED)  # [K,P,ED]
        aut2 = np.zeros((128, K, 2, P), np.float32)
        aut2[:ED, :, 0, :] = au_c.transpose(2, 0, 1)
        aut2[ED, :, 0, :] = 1.0
        aut2 = aut2.astype(FP8)

        # gi_n: n-gate input contribution (+ bih n-part), host-precomputed,
        # k-major in DRAM so per-step slices are independent DMAs:
        # gin[k, p, ct, j] = (au_c[k, j] @ wih_n.T + bih_n)[ct*128 + p]
        gi = au_c @ wih[2 * H:].T + bih[2 * H:]                     # [K,P,H]
        gin = np.ascontiguousarray(
            gi.transpose(0, 2, 1).reshape(K, HKC, 128, P)
            .transpose(0, 2, 1, 3)).astype(BF16)                   # [K,128,4,P]

        # ftt: [128, 4, PF] in consumption (fi) order
        ft_c = ft_full[sl][:, forder].transpose(3, 1, 0, 2).reshape(H, PF)
        ftt = np.ascontiguousarray(
            ft_c.reshape(HKC, 128, PF).transpose(1, 0, 2)).astype(FP8)

        # masks, fi-ordered position flat index = fi*P + i*TS + s;
        # stored in the per-fi 128-row tail layout
        posflat = np.ascontiguousarray(
            maskf[sl][:, forder].transpose(1, 0, 2)).reshape(PF)    # [768]
        negflat = np.repeat(posflat, NNEG)                          # [15360]
        mskp = np.ascontiguousarray(
            posflat.reshape(FS, 128, PH // 128).transpose(1, 0, 2)
            .reshape(128, PF // 128)).astype(BF16)
        mskn = np.ascontiguousarray(
            negflat.reshape(FS, 128, NH // 128).transpose(1, 0, 2)
            .reshape(128, NSLOT // 128)).astype(BF16)
        denoms.append(float(posflat.sum()))

        # negatives: host gather + transpose, chunk-major fp8
        # negt[p, m, i, s] = pool8[v[m*SC+s], i*128+p]
        v = np.concatenate([negi4[f, sl].reshape(-1) for f in forder])
        g = pool8[v]                                                # [NSLOT,512]
        negt = np.ascontiguousarray(
            g.reshape(NSC, SC, HKC, 128).transpose(3, 0, 2, 1))     # [128,NSC,4,SC]

        in_maps.append(dict(
            whhc=whhc, aut2=np.ascontiguousarray(aut2),
            ht08=ht08,
            w1ap=w1ap, w1bp=w1bp, w2p=w2p, w3p=w3p,
            b1t=b1_dev, b2t=b2_dev, b3c=b3c,
            ftt=ftt, negt=negt, mskn=mskn, mskp=mskp,
            gin=gin,
        ))

    return in_maps, tuple(int(u) for u in usub), sum(denoms)


# ----------------------------------------------------------------------------
# device program
# ----------------------------------------------------------------------------

def _build(usub_vals):
    import concourse.bass as bass
    import concourse.bacc as bacc
    import concourse.mybir as mybir
    import concourse.tile as tile

    dt = mybir.dt
    AF = mybir.ActivationFunctionType
    AL = mybir.AluOpType
    DR = mybir.MatmulPerfMode.DoubleRow
    RELU_ADD = _relu_add_op()

    forder = sorted(range(FS), key=lambda f: (usub_vals[f], f))
    parkn = min(NSC, 23)

    nc = bacc.Bacc("TRN2", target_bir_lowering=False, debug=False,
                   num_devices=NCORE)

    def din(name, shape, d):
        return nc.dram_tensor(name, shape, d, kind="ExternalInput").ap()

    whhc = din("whhc", [128, 3, 2, 3 * H], dt.float8e4)
    aut2 = din("aut2", [128, K, 2, P], dt.float8e4)
    ht08 = din("ht08", [128, HKC, P], dt.float8e4)
    w1ap = din("w1ap", [128, 2, 2, H], dt.float8e4)
    w1bp = din("w1bp", [128, 2, 2, H], dt.float8e4)
    w2p = din("w2p", [128, 2, 2, H], dt.float8e4)
    w3p = din("w3p", [128, 2, 2, 128], dt.float8e4)
    b1t = din("b1t", [128, HKC], dt.float32)
    b2t = din("b2t", [128, HKC], dt.float32)
    b3c = din("b3c", [128, 2], dt.float32)
    ftt = din("ftt", [128, HKC, PF], dt.float8e4)
    gind = din("gin", [K, 128, HKC, P], dt.bfloat16)
    negd = din("negt", [128, NSC, HKC, SC], dt.float8e4)
    msknd = din("mskn", [128, NSLOT // 128], dt.bfloat16)
    mskpd = din("mskp", [128, PF // 128], dt.bfloat16)
    out = nc.dram_tensor("out", [1, 4], dt.float32, kind="ExternalOutput").ap()

    with tile.TileContext(nc) as tc:
        with (
            tc.tile_pool(name="cw", bufs=1) as cw,
            tc.tile_pool(name="ps2", bufs=6, space="PSUM") as ps2,
            tc.tile_pool(name="plp", bufs=2, space="PSUM") as plp,
            tc.tile_pool(name="ng", bufs=5) as ng,
            tc.tile_pool(name="lr", bufs=4) as lr,
            tc.tile_pool(name="dsc", bufs=1, space="DRAM") as dsc,
        ):
            # logit rows land here via a small SBUF staging row (bf16)
            dROW = dsc.tile([FS, NH + PH], dt.bfloat16, name="drow")

            # ---------------- DMA: priority order ----------------
            # group A: GRU-critical (scan starts as soon as these land)
            tWHH = cw.tile([128, 3, 2, 3 * H], dt.float8e4, tag="whhc",
                           name="whhc")
            tC8 = [cw.tile([128, HKC, P], dt.float8e4, tag=f"c8{i}",
                           name=f"c8{i}") for i in range(2)]
            tAUT = cw.tile([128, K, 2, P], dt.float8e4, tag="aut2",
                           name="aut2")
            tGIN = cw.tile([128, K, HKC, P], dt.bfloat16, tag="gin",
                           name="gin")
            nc.sync.dma_start(out=tWHH[:, 0], in_=whhc[:, 0])
            nc.sync.dma_start(out=tC8[0][:], in_=ht08[:])
            nc.sync.dma_start(out=tWHH[:, 2], in_=whhc[:, 2])
            nc.sync.dma_start(out=tAUT[:, 0:4], in_=aut2[:, 0:4])
            nc.sync.dma_start(out=tWHH[:, 1], in_=whhc[:, 1])
            nc.sync.dma_start(out=tGIN[:, 0], in_=gind[0])
            nc.sync.dma_start(out=tAUT[:, 4:], in_=aut2[:, 4:])

            # group B: streamed in consumption order (in-order DMA queue)
            tNEG = cw.tile([128, NSC, HKC, SC], dt.float8e4, tag="negt",
                           name="negt")
            tW1A = cw.tile([128, 2, 2, H], dt.float8e4, tag="w1ap", name="w1ap")
            tW1B = cw.tile([128, 2, 2, H], dt.float8e4, tag="w1bp", name="w1bp")
            tW2 = cw.tile([128, 2, 2, H], dt.float8e4, tag="w2p", name="w2p")
            tW3 = cw.tile([128, 2, 2, 128], dt.float8e4, tag="w3p", name="w3p")
            tB1 = cw.tile([128, HKC], dt.float32, tag="b1t", name="b1t")
            tB2 = cw.tile([128, HKC], dt.float32, tag="b2t", name="b2t")
            tB3C = cw.tile([128, 2], dt.float32, tag="b3c", name="b3c")
            tFTT = cw.tile([128, HKC, PF], dt.float8e4, tag="ftt", name="ftt")
            tMSKN = cw.tile([128, NSLOT // 128], dt.bfloat16, tag="mskn",
                            name="mskn")
            tMSKP = cw.tile([128, PF // 128], dt.bfloat16, tag="mskp",
                            name="mskp")

            nc.sync.dma_start(out=tNEG[:, 0:2], in_=negd[:, 0:2])
            nc.sync.dma_start(out=tGIN[:, 1], in_=gind[1])
            nc.sync.dma_start(out=tNEG[:, 2:4], in_=negd[:, 2:4])
            nc.sync.dma_start(out=tGIN[:, 2], in_=gind[2])
            nc.sync.dma_start(out=tW1A[:], in_=w1ap[:])
            nc.sync.dma_start(out=tW1B[:], in_=w1bp[:])
            nc.sync.dma_start(out=tB1[:], in_=b1t[:])
            nc.sync.dma_start(out=tNEG[:, 4:6], in_=negd[:, 4:6])
            nc.sync.dma_start(out=tGIN[:, 3], in_=gind[3])
            nc.sync.dma_start(out=tNEG[:, 6:8], in_=negd[:, 6:8])
            nc.sync.dma_start(out=tGIN[:, 4], in_=gind[4])
            nc.sync.dma_start(out=tW2[:], in_=w2p[:])
            nc.sync.dma_start(out=tW3[:], in_=w3p[:])
            nc.sync.dma_start(out=tB2[:], in_=b2t[:])
            nc.sync.dma_start(out=tB3C[:], in_=b3c[:])
            nc.sync.dma_start(out=tNEG[:, 8:10], in_=negd[:, 8:10])
            nc.sync.dma_start(out=tGIN[:, 5], in_=gind[5])
            nc.sync.dma_start(out=tNEG[:, 10:12], in_=negd[:, 10:12])
            nc.sync.dma_start(out=tGIN[:, 6], in_=gind[6])
            nc.sync.dma_start(out=tFTT[:], in_=ftt[:])
            nc.sync.dma_start(out=tGIN[:, 7], in_=gind[7])
            nc.sync.dma_start(out=tNEG[:, 12:16], in_=negd[:, 12:16])
            nc.sync.dma_start(out=tNEG[:, 16:20], in_=negd[:, 16:20])
            nc.sync.dma_start(out=tNEG[:, 20:24], in_=negd[:, 20:24])
            nc.sync.dma_start(out=tNEG[:, 24:28], in_=negd[:, 24:28])
            nc.sync.dma_start(out=tNEG[:, 28:32], in_=negd[:, 28:32])
            nc.sync.dma_start(out=tMSKN[:], in_=msknd[:])
            nc.sync.dma_start(out=tMSKP[:], in_=mskpd[:])

            # persistent state tiles
            tAT = cw.tile([128, HKC, PF], dt.bfloat16, tag="at", name="at")
            tR = cw.tile([128, HKC, P], dt.bfloat16, tag="r", name="r")
            tZ = cw.tile([128, HKC, P], dt.bfloat16, tag="z", name="z")
            tLV = cw.tile([128, NSLOT // 128], dt.bfloat16, tag="lv", name="lv")
            tLPV = cw.tile([128, PF // 128], dt.bfloat16, tag="lpv", name="lpv")
            tAN = cw.tile([128, 8], dt.float32, tag="an", name="an")
            tONE = cw.tile([128, 1], dt.float32, tag="one", name="one")
            nc.vector.memset(tONE[:], 1.0)
            tRES = cw.tile([1, 4], dt.float32, tag="res", name="res")

            # ---------------- negative-chunk helpers ----------------
            tPARK = cw.tile([128, max(parkn, 1), HKC, SC], dt.float8e4,
                            tag="park", name="park")

            def l1_matmuls(m, dst4):
                """W1b @ negt chunk m into four 1-bank PSUM tiles."""
                for ht in range(HKC):
                    p1 = ps2.tile([128, 512], dt.float32, tag="ps")
                    for g in range(2):
                        nc.tensor.matmul(
                            p1[:, :SC],
                            lhsT=tW1B[:, g, :, ht * 128:(ht + 1) * 128],
                            rhs=tNEG[:, m, 2 * g:2 * g + 2, :],
                            start=(g == 0), stop=(g == 1), perf_mode=DR)
                    dst4[ht] = p1

            # ---- per-fi tail: DRAM bounces per piece (8/4/4 chunks); the
            # softplus block runs in two stages so only the tiny last piece
            # is serialized after the final chunk ----
            POFF = (0, NH // 2, 3 * NH // 4, NH)

            def bounce_piece(o, pc):
                c0, c1 = POFF[pc], POFF[pc + 1]
                nc.sync.dma_start(
                    out=tLV[:, (o * NH + c0) // 128:(o * NH + c1) // 128],
                    in_=dROW[o:o + 1, c0:c1]
                    .rearrange("a (p c) -> (a p) c", p=128))

            def bounce_pos(o):
                pcol = PH // 128
                nc.sync.dma_start(
                    out=tLPV[:, o * pcol:(o + 1) * pcol],
                    in_=dROW[o:o + 1, NH:]
                    .rearrange("a (p c) -> (a p) c", p=128))

            def tail_stage1():
                # everything except the last piece of the last fi; exps
                # grouped before lns so each act table loads exactly once
                w = (NSLOT - NH // 4) // 128
                nc.scalar.activation(out=tLV[:, :w], in_=tLV[:, :w],
                                     func=AF.Exp, bias=tB3C[:, 0:1])
                nc.scalar.activation(out=tLPV[:], in_=tLPV[:], func=AF.Exp,
                                     scale=-1.0, bias=tB3C[:, 1:2])
                nc.vector.tensor_mul(out=tLV[:, :w], in0=tLV[:, :w],
                                     in1=tMSKN[:, :w])
                nc.vector.tensor_mul(out=tLPV[:], in0=tLPV[:], in1=tMSKP[:])
                nc.scalar.activation(out=tLV[:, :w], in_=tLV[:, :w],
                                     func=AF.Ln, bias=1.0,
                                     accum_out=tAN[:, 1:2])
                nc.scalar.activation(out=tLPV[:], in_=tLPV[:], func=AF.Ln,
                                     bias=1.0, accum_out=tAN[:, 0:1])

            def tail_stage2():
                # rows already hold exp(x+b3); only mul + ln remain and the
                # ln table is already loaded
                w = (NSLOT - NH // 4) // 128
                nc.vector.tensor_mul(out=tLV[:, w:], in0=tLV[:, w:],
                                     in1=tMSKN[:, w:])
                nc.scalar.activation(out=tLV[:, w:], in_=tLV[:, w:],
                                     func=AF.Ln, bias=1.0,
                                     accum_out=tAN[:, 2:3])
                nc.vector.tensor_add(out=tAN[:, 1:2], in0=tAN[:, 1:2],
                                     in1=tAN[:, 2:3])
                for col, srcc in ((0, 0), (1, 1)):
                    pr = plp.tile([128, 512], dt.float32, tag="pl",
                                  name="pr")
                    nc.tensor.matmul(pr[:1, :1],
                                     lhsT=tAN[:, srcc:srcc + 1],
                                     rhs=tONE[:], start=True, stop=True)
                    nc.vector.tensor_copy(out=tRES[0:1, col:col + 1],
                                          in_=pr[:1, :1])
                nc.vector.memset(tRES[0:1, 2:4], 0.0)
                nc.sync.dma_start(out=out[:], in_=tRES[:])

            # ---------------- per-f section (generator) ----------------
            # fi below is the consumption-order position (0 = first ready);
            # chunk indices, tAT/ftt/mask columns all use this ordering.
            def emit_f_section(fi, n8):
                cols = slice(fi * P, (fi + 1) * P)
                # AT = W1a @ fp + b1  (fp = n8) — MUST run at its step (n8
                # is recycled two steps later)
                for ht in range(HKC):
                    p1 = ps2.tile([128, 512], dt.float32, tag="ps")
                    for g in range(2):
                        nc.tensor.matmul(
                            p1[:, :P],
                            lhsT=tW1A[:, g, :, ht * 128:(ht + 1) * 128],
                            rhs=n8[:, 2 * g:2 * g + 2, :],
                            start=(g == 0), stop=(g == 1), perf_mode=DR)
                    nc.scalar.activation(
                        out=tAT[:, ht, cols], in_=p1[:, :P],
                        func=AF.Identity, bias=tB1[:, ht:ht + 1])
                yield
                # positives: h1 = relu(W1b@ft + AT); h2 = relu(W2@h1+b2)
                h1 = ng.tile([128, HKC, P], dt.float8e4, tag="h1", name="h1p")
                for ht in range(HKC):
                    p1 = ps2.tile([128, 512], dt.float32, tag="ps")
                    for g in range(2):
                        nc.tensor.matmul(
                            p1[:, :P],
                            lhsT=tW1B[:, g, :, ht * 128:(ht + 1) * 128],
                            rhs=tFTT[:, 2 * g:2 * g + 2, cols],
                            start=(g == 0), stop=(g == 1), perf_mode=DR)
                    nc.vector._custom_dve(
                        RELU_ADD, out=h1[:, ht, :], in0=p1[:, :P],
                        in1=tAT[:, ht, cols])
                yield
                h2 = ng.tile([128, HKC, P], dt.float8e4, tag="h2", name="h2p")
                for ht in range(HKC):
                    p1 = ps2.tile([128, 512], dt.float32, tag="ps")
                    for g in range(2):
                        nc.tensor.matmul(
                            p1[:, :P],
                            lhsT=tW2[:, g, :, ht * 128:(ht + 1) * 128],
                            rhs=h1[:, 2 * g:2 * g + 2, :],
                            start=(g == 0), stop=(g == 1), perf_mode=DR)
                    nc.scalar.activation(
                        out=h2[:, ht, :], in_=p1[:, :P],
                        func=AF.Relu, bias=tB2[:, ht:ht + 1])
                pl = plp.tile([128, 512], dt.float32, tag="pl")
                for g in range(2):
                    nc.tensor.matmul(
                        pl[:, :P], lhsT=tW3[:, g],
                        rhs=h2[:, 2 * g:2 * g + 2, :],
                        start=(g == 0), stop=(g == 1), perf_mode=DR)
                row = lr.tile([1, SC], dt.bfloat16, tag="lrow", name="rowp")
                nc.scalar.activation(out=row[0:1, :P], in_=pl[0:1, :P],
                                     func=AF.Identity)
                nc.sync.dma_start(out=dROW[fi:fi + 1, NH:],
                                  in_=row[0:1, :P])
                bounce_pos(fi)
                yield
                # negatives, software-pipelined: L1(m) | L2(m-1) | L3(m-2)
                h1s, h2s = {}, {}
                for m in range(SCF + 2):
                    if m < SCF:
                        cm = fi * SCF + m
                        a0 = fi * P + m * 24
                        h1n = ng.tile([128, HKC, SC], dt.float8e4, tag="h1",
                                      name="h1n")
                        if cm < parkn:
                            for ht in range(HKC):
                                nc.vector._custom_dve(
                                    RELU_ADD,
                                    out=h1n[:, ht, :].rearrange(
                                        "p (a b) -> p a b", b=NNEG),
                                    in0=tPARK[:, cm, ht, :].rearrange(
                                        "p (a b) -> p a b", b=NNEG),
                                    in1=tAT[:, ht, a0:a0 + 24][:, :, None]
                                    .broadcast_to((128, 24, NNEG)))
                        else:
                            p4 = [None] * HKC
                            l1_matmuls(cm, p4)
                            for ht in range(HKC):
                                nc.vector._custom_dve(
                                    RELU_ADD,
                                    out=h1n[:, ht, :].rearrange(
                                        "p (a b) -> p a b", b=NNEG),
                                    in0=p4[ht][:, :SC].rearrange(
                                        "p (a b) -> p a b", b=NNEG),
                                    in1=tAT[:, ht, a0:a0 + 24][:, :, None]
                                    .broadcast_to((128, 24, NNEG)))
                        h1s[m] = h1n
                    if 1 <= m <= SCF:
                        h1n = h1s.pop(m - 1)
                        h2n = ng.tile([128, HKC, SC], dt.float8e4, tag="h2",
                                      name="h2n")
                        for ht in range(HKC):
                            p1 = ps2.tile([128, 512], dt.float32, tag="ps")
                            for g in range(2):
                                nc.tensor.matmul(
                                    p1[:, :SC],
                                    lhsT=tW2[:, g, :,
                                             ht * 128:(ht + 1) * 128],
                                    rhs=h1n[:, 2 * g:2 * g + 2, :],
                                    start=(g == 0), stop=(g == 1),
                                    perf_mode=DR)
                            nc.scalar.activation(
                                out=h2n[:, ht, :], in_=p1[:, :SC],
                                func=AF.Relu, bias=tB2[:, ht:ht + 1])
                        h2s[m - 1] = h2n
                    if m >= 2:
                        ml = m - 2
                        h2n = h2s.pop(ml)
                        pl = plp.tile([128, 512], dt.float32, tag="pl")
                        for g in range(2):
                            nc.tensor.matmul(
                                pl[:, :SC], lhsT=tW3[:, g],
                                rhs=h2n[:, 2 * g:2 * g + 2, :],
                                start=(g == 0), stop=(g == 1), perf_mode=DR)
                        row = lr.tile([1, SC], dt.bfloat16, tag="lrow",
                                      name="rown")
                        if fi == FS - 1 and ml >= 3 * SCF // 4:
                            # last piece: store exp(x+b3) so the final stage
                            # needs only the ln table (no extra table swap)
                            nc.scalar.activation(out=row[:], in_=pl[0:1, :SC],
                                                 func=AF.Exp,
                                                 bias=tB3C[0:1, 0:1])
                        elif ml % 2 == 0:
                            nc.vector.tensor_copy(out=row[:], in_=pl[0:1, :SC])
                        else:
                            nc.scalar.activation(out=row[:], in_=pl[0:1, :SC],
                                                 func=AF.Identity)
                        nc.sync.dma_start(
                            out=dROW[fi:fi + 1, ml * SC:(ml + 1) * SC],
                            in_=row[:])
                    if m - 2 == SCF // 2 - 1:
                        bounce_piece(fi, 0)
                    elif m - 2 == 3 * SCF // 4 - 1:
                        bounce_piece(fi, 1)
                    if fi == FS - 1 and m == SCF + 1:
                        # Scalar is past its last relu; exp/ln are grouped so
                        # each table loads once, hidden inside the drain
                        tail_stage1()
                    yield
                bounce_piece(fi, 2)
                if fi == FS - 1:
                    tail_stage2()

            def park_chunk(m):
                """L1 matmuls for chunk m + pre-activation parked to SBUF
                fp8 (copies split DVE/Scalar; AT-add applied post-scan).
                Bridges the per-step chain-tail PE gap so the PE pstate
                stays ramped."""
                p4 = [None] * HKC
                l1_matmuls(m, p4)
                for ht in range(HKC):
                    eng = nc.vector.tensor_copy if ht % 2 == 0 else None
                    if eng:
                        eng(out=tPARK[:, m, ht, :], in_=p4[ht][:, :SC])
                    else:
                        nc.scalar.activation(out=tPARK[:, m, ht, :],
                                             in_=p4[ht][:, :SC],
                                             func=AF.Identity)

            # ---------------- GRU scan + interleaving ----------------
            parked = [0]
            pending = []
            for k in range(K):
                c8, n8 = tC8[k % 2], tC8[(k + 1) % 2]

                # Each matmul group is split: kt01+aut passes depend only on
                # the previous step's cp0 state half, kt23 on cp1.  Emitting
                # all cp0-dependent passes first lets the PE bridge the
                # previous step's chain tail.
                def open_grp(gts, tag):
                    ps = []
                    for gt in gts:
                        p1 = ps2.tile([128, 512], dt.float32, tag="ps")
                        nc.tensor.matmul(
                            p1[:, :P],
                            lhsT=tWHH[:, 0, :, gt * 128:(gt + 1) * 128],
                            rhs=c8[:, 0:2, :],
                            start=True, stop=False, perf_mode=DR)
                        nc.tensor.matmul(
                            p1[:, :P],
                            lhsT=tWHH[:, 2, :, gt * 128:(gt + 1) * 128],
                            rhs=tAUT[:, k],
                            start=False, stop=False, perf_mode=DR)
                        ps.append(p1)
                    return ps

                def close_grp(ps, gts):
                    for p1, gt in zip(ps, gts):
                        nc.tensor.matmul(
                            p1[:, :P],
                            lhsT=tWHH[:, 1, :, gt * 128:(gt + 1) * 128],
                            rhs=c8[:, 2:4, :],
                            start=False, stop=True, perf_mode=DR)

                def sig(ps, dst, cp, split=False):
                    for j in range(2):
                        nc.scalar.activation(
                            out=dst[:, cp + j, :], in_=ps[j][:, :P],
                            func=AF.Sigmoid)

                def chain_a(cp, phs):
                    # per-j (single ct tile) ops halve each link's latency
                    # on the serial path; j1 trails j0 by one engine slot
                    t2 = ng.tile([128, 2, P], dt.bfloat16, tag="tm", name="t2")
                    c2 = ng.tile([128, 2, P], dt.bfloat16, tag="tm", name="c2")
                    d2 = ng.tile([128, 2, P], dt.bfloat16, tag="tm", name="d2")
                    for j in range(2):
                        ct = cp * 2 + j
                        # psum = gh_n + bhh_n (bias row in the aut pass), so
                        # the r-mult is a plain tensor_tensor
                        nc.vector.tensor_mul(out=t2[:, j, :],
                                             in0=phs[j][:, :P],
                                             in1=tR[:, ct, :])
                        nc.vector.tensor_add(out=t2[:, j, :], in0=t2[:, j, :],
                                             in1=tGIN[:, k, ct, :])
                        nc.scalar.activation(out=c2[:, j, :], in_=t2[:, j, :],
                                             func=AF.Tanh)
                        nc.vector.tensor_sub(out=d2[:, j, :],
                                             in0=c8[:, ct, :],
                                             in1=c2[:, j, :])
                    return c2, d2

                def chain_b(cp, c2, d2):
                    for j in range(2):
                        ct = cp * 2 + j
                        nc.vector.tensor_mul(out=d2[:, j, :], in0=d2[:, j, :],
                                             in1=tZ[:, ct, :])
                        # fp8 state: the final add writes the next-step state
                        # tile directly (no separate bf16 state / cast)
                        nc.vector.tensor_add(out=n8[:, ct, :],
                                             in0=d2[:, j, :],
                                             in1=c2[:, j, :])

                if k >= 1 and parked[0] < parkn:
                    park_chunk(parked[0]); parked[0] += 1
                # phase 1: cp0-dependent partials for r01, r23, n0
                pA = open_grp((0, 1), "r01")
                pB = open_grp((2, 3), "r23")
                pC = open_grp((8, 9), "n0")
                # phase 2: cp1-dependent closes; n0 closes right after the
                # r01 sigmoid so the cp0 chain launches ASAP
                close_grp(pA, (0, 1))
                sig(pA, tR, 0, split=True)
                close_grp(pC, (8, 9))
                ca0 = chain_a(0, pC)
                close_grp(pB, (2, 3))
                sig(pB, tR, 2, split=True)
                # n1 full group
                pD = []
                for gt in (10, 11):
                    p1 = ps2.tile([128, 512], dt.float32, tag="ps")
                    nc.tensor.matmul(
                        p1[:, :P],
                        lhsT=tWHH[:, 0, :, gt * 128:(gt + 1) * 128],
                        rhs=c8[:, 0:2, :],
                        start=True, stop=False, perf_mode=DR)
                    nc.tensor.matmul(
                        p1[:, :P],
                        lhsT=tWHH[:, 2, :, gt * 128:(gt + 1) * 128],
                        rhs=tAUT[:, k],
                        start=False, stop=False, perf_mode=DR)
                    nc.tensor.matmul(
                        p1[:, :P],
                        lhsT=tWHH[:, 1, :, gt * 128:(gt + 1) * 128],
                        rhs=c8[:, 2:4, :],
                        start=False, stop=True, perf_mode=DR)
                    pD.append(p1)
                pE = open_grp((4, 5), "z01")
                close_grp(pE, (4, 5))
                sig(pE, tZ, 0)
                chain_b(0, *ca0)
                ca1 = chain_a(1, pD)
                pF = open_grp((6, 7), "z23")
                close_grp(pF, (6, 7))
                sig(pF, tZ, 2)
                chain_b(1, *ca1)
                                # chain-tail filler: parked L1 chunks keep the PE busy
                # (and its pstate ramped) across the serial chain tail
                if parked[0] + 2 <= parkn:
                    park_chunk(parked[0])
                    park_chunk(parked[0] + 1)
                    parked[0] += 2
                for fi in range(FS):
                    if usub_vals[forder[fi]] == k:
                        sec = emit_f_section(fi, n8)
                        next(sec)           # AT stage now; rest post-scan
                        pending.append(sec)
                if k == K - 1:
                    while pending:
                        try:
                            next(pending[0])
                        except StopIteration:
                            pending.pop(0)

    nc.compile()
    return nc


def _get_program(usub_vals):
    key = usub_vals
    if key not in _PROG_CACHE:
        _PROG_CACHE[key] = _build(usub_vals)
    return _PROG_CACHE[key]


def kernel(**inputs):
    from concourse.bass_utils import run_bass_kernel_spmd
    in_maps, usub_vals, denom = _prep(inputs)
    nc = _get_program(usub_vals)
    res = run_bass_kernel_spmd(nc, in_maps, list(range(NCORE)))
    parts = np.stack([np.asarray(res.results[c]['out'][0], np.float64)
                      for c in range(NCORE)])
    pos, neg = parts[:, 0].sum(), parts[:, 1].sum()
    return np.float32(0.1 * (pos / denom + neg / (denom * NNEG)))


# revision 42
# speedup vs baseline: 1.0111x; 1.0111x over previous
"""Trainium2 Bass kernel for nn_CPCA (CPC-action loss).

Strategy: data-parallel over the env dim n (64 envs/core on 8 cores).
v4 (from the v3 indirect-gather baseline, 319us -> ~239us):
  - negatives are gathered AND transposed on the host into a chunk-major
    fp8 tensor streamed to SBUF by plain DMA.  This removes v3's 120
    serialized indirect DMAs (~1.1us each), 240 PE transposes and 120
    PSUM->SBUF copies that rate-limited the first 120us.
  - DMA issue order prioritized: GRU-critical tensors first (scan starts
    ~10us in), negatives/gin/weights stream in consumption order.
  - all PSUM tiles are single-bank [128,512] with a 6-deep rotation;
    the finer recycle granularity removes most PE psum-wait stalls.
  - GRU step emission is split by state-half dependency: kt01+aut
    partial-accumulations (needing only the previous step's first
    state half) issue before the kt23 closes, so the PE bridges the
    serial chain tail; r-gates and n-gates go first, z-gates ride in
    the chain's shadow.  The state stays in fp8 (tC8 tiles) and the
    update chain runs per-ct-tile to halve each link's latency.
  - bhh_n rides the n-matmul bias row so the r-mult is a plain wide
    tensor_tensor; gi_n (+bih_n) is host-precomputed (gin).
  - L1 matmuls for 21 of the 32 negative chunks run during the scan as
    PE filler ("parking": pre-activation copied to SBUF fp8, the
    relu(x+AT) applied post-scan) keeping the PE pstate ramped.
  - sections (AT/pos/neg pipelines) drain post-scan, software-pipelined
    L1(m)|L2(m-1)|L3(m-2); only each fi's AT stage runs at its step
    (its n8 state tile is recycled two steps later).
  - logit rows go PSUM -> small SBUF row -> DRAM, bounced back per
    8/4/4-chunk piece into a 128-row layout; softplus accumulation runs
    in two stages with exp/ln grouped so the act table loads exactly
    once per function family, overlapping the drain.
Per-core partial sums (pos_loss_sum, neg_loss_sum) are combined with
the host-side mask count into the scalar loss.
"""
import sys

if '/opt/trn_rl_repo' not in sys.path:
    sys.path.insert(0, '/opt/trn_rl_repo')

import numpy as np
import ml_dtypes

BF16 = ml_dtypes.bfloat16
FP8 = ml_dtypes.float8_e4m3   # IEEE e4m3 (max 240) == TRN fp8_exp4

N, T, H, TS, FS, K, A, ED, NNEG = 512, 128, 512, 6, 2, 8, 17, 32, 20
NCORE = 8
NE = N // NCORE          # 64 envs per core
P = NE * TS              # 384 positions per core (per unroll index)
PF = FS * P              # 768
NSLOT = FS * P * NNEG    # 15360 negative slots per core
SC = 24 * NNEG           # 480 slots (24 positions) per matmul sub-chunk
NSC = NSLOT // SC        # 32 sub-chunks (16 per unroll index)
SCF = NSC // FS          # 16
HKC = H // 128           # 4
NH = NSLOT // FS         # 7680 neg logits per fi
PH = P                   # 384 pos logits per fi

_PROG_CACHE = {}


# ----------------------------------------------------------------------------
# custom DVE op: out = relu(in0 + in1)   (in1 may be a stride-0 broadcast)
# ----------------------------------------------------------------------------

def _relu_add_op():
    from concourse import dve_ops
    from concourse.dve_spec import Spec, Src0, Src1, relu, lower
    from concourse.dve_uop import DveOpSpec

    name = "RELU_ADD_CPCA"
    for op in dve_ops.OPS:
        if op.name == name:
            return op

    def _ref(in0, in1, c0, c1, c2):
        x = np.asarray(in0, np.float32) + \
            np.asarray(in1, np.float32).reshape(np.asarray(in0).shape)
        return np.maximum(
            np.nan_to_num(x, nan=0.0, posinf=np.inf, neginf=-np.inf), 0)

    spec = Spec(body=relu(Src0 + Src1), reference=_ref)
    shas = {}
    for ver in ("v3", "v4"):
        tmp = DveOpSpec(name=name, opcode=31, uops=lower(spec, ver=ver),
                        rd1_en=True)
        shas[ver] = tmp.sha(ver)
    op = dve_ops.DveOp(name, spec, subdim=False, uops_sha=shas)
    dve_ops.OPS.append(op)
    dve_ops.CUSTOM_DVE_SPECS[name] = spec
    dve_ops._SUB_OPCODE_FOR_NAME[name] = (
        dve_ops._CUSTOM_DVE_ROW_BASE + len(dve_ops.OPS) - 1)
    assert dve_ops._SUB_OPCODE_FOR_NAME[name] < 0x20
    return op


# ----------------------------------------------------------------------------
# host-side input preparation (sharding / layout / index metadata only)
# ----------------------------------------------------------------------------

def _prep(inputs):
    acts = np.asarray(inputs['actions']).astype(np.int64)          # [N,T,1]
    nd = np.asarray(inputs['not_dones'], np.float32)               # [N,T,1]
    vld = np.asarray(inputs['valids']).astype(bool)                # [N,T,1]
    ri = np.asarray(inputs['rnn_inputs'], np.float32)              # [N,T,H]
    ro = np.asarray(inputs['rnn_outputs'], np.float32)             # [N,T,H]
    embw = np.asarray(inputs['embed_w'], np.float32)               # [A,ED]
    wih = np.asarray(inputs['gru_w_ih'], np.float32)               # [3H,ED]
    whh = np.asarray(inputs['gru_w_hh'], np.float32)               # [3H,H]
    bih = np.asarray(inputs['gru_b_ih'], np.float32)               # [3H]
    bhh = np.asarray(inputs['gru_b_hh'], np.float32)               # [3H]
    w1 = np.asarray(inputs['p_w1'], np.float32)                    # [H,2H]
    b1 = np.asarray(inputs['p_b1'], np.float32)                    # [H]
    w2 = np.asarray(inputs['p_w2'], np.float32)                    # [H,H]
    b2 = np.asarray(inputs['p_b2'], np.float32)                    # [H]
    w3 = np.asarray(inputs['p_w3'], np.float32)                    # [1,H]
    b3 = np.asarray(inputs['p_b3'], np.float32)                    # [1]
    tsub = np.asarray(inputs['time_subsample']).astype(np.int64)   # [TS]
    usub = np.asarray(inputs['unroll_subsample']).astype(np.int64) # [FS]
    negi = np.asarray(inputs['neg_indices']).astype(np.int64)      # [FS*TS*N*NNEG]
    maxk = int(np.asarray(inputs['max_k']))
    assert maxk == K, maxk
    assert tsub.shape == (TS,) and usub.shape == (FS,)

    forder = np.argsort(usub, kind='stable')                       # consumption order

    # ---- shared (replicated) tensors -------------------------------------
    def dr_std(w):
        # [p, g, i, m] = w[m, (2g+i)*128 + p]
        return np.ascontiguousarray(
            w.T.reshape(2, 2, 128, -1).transpose(2, 0, 1, 3)).astype(FP8)

    # GRU combined lhsT: k-tiles 0-3 = whh.T chunks, 4 = wih.T (+bias row 32),
    # 5 = zeros.  Gates r/z get bih+bhh via the bias row; the n-gate columns
    # of the aut plane carry ONLY bhh_n in the bias row (wih_n zeroed - the
    # ih part arrives host-precomputed via gin), so the n psum is directly
    # gh_n + bhh_n and the r-mult is a plain tensor_tensor multiply.
    wk = np.zeros((6, 128, 3 * H), np.float32)
    wk[:4] = whh.T.reshape(4, 128, 3 * H)
    wk[4, :ED, :2 * H] = wih.T[:, :2 * H]
    wk[4, ED] = np.concatenate([(bih + bhh)[:2 * H], bhh[2 * H:]])
    whhc = np.ascontiguousarray(
        wk.reshape(3, 2, 128, 3 * H).transpose(2, 0, 1, 3)).astype(FP8)

    w1ap = dr_std(w1[:, :H])
    w1bp = dr_std(w1[:, H:])
    w2p = dr_std(w2)
    # layer-3 weights broadcast to all 128 output rows — skinny (M<4) DR
    # ldweights fail the walrus ISA check; only PSUM partition 0 is read.
    w3p = np.ascontiguousarray(np.broadcast_to(
        w3[0].reshape(2, 2, 128).transpose(2, 0, 1)[..., None],
        (128, 2, 2, 128))).astype(FP8)

    b1_dev = np.ascontiguousarray(b1.reshape(HKC, 128).T)
    b2_dev = np.ascontiguousarray(b2.reshape(HKC, 128).T)
    b3c = np.broadcast_to(np.array([b3[0], -b3[0]], np.float32), (128, 2)).copy()

    # negatives pool in fp8 (same values the v3 device path consumed)
    pool8 = ri.reshape(N * T, H).astype(FP8)

    # ---- per-core views ---------------------------------------------------
    ks = np.arange(K)
    tq = tsub[None, :] + ks[:, None]                                # [K,TS]
    ok_au = tq <= T - 2
    a_idx = acts[:, np.clip(tq, 0, T - 1), 0]                       # [N,K,TS]
    au_full = embw[a_idx] * ok_au[None, :, :, None]                 # [N,K,TS,ED]

    tf = tsub[None, :] + usub[:, None]                              # [FS,TS]
    ok_ft = tf <= T - 2
    ft_full = np.where(ok_ft[None, :, :, None],
                       ri[:, np.clip(tf + 1, 0, T - 1)], 0.0)       # [N,FS,TS,H]

    vm = ((nd[:, :, 0] > 0) & vld[:, :, 0]).astype(np.float32)      # [N,T]
    vmk = np.where(ok_au[None], vm[:, np.clip(tq, 0, T - 1)], 0.0)  # [N,K,TS]
    cum = np.cumprod(vmk, axis=1)                                   # [N,K,TS]
    maskf = cum[:, usub, :]                                         # [N,FS,TS]

    negi4 = negi.reshape(FS, N, TS, NNEG)

    in_maps = []
    denoms = []
    for c in range(NCORE):
        sl = slice(c * NE, (c + 1) * NE)

        # h0: [128, 4, P] dev[p,kc,j] = ro[i, ts_s, kc*128+p], j = i*TS+s
        h0 = ro[sl][:, tsub].reshape(P, H).T                        # [H,P]
        ht0 = np.ascontiguousarray(h0.reshape(HKC, 128, P).transpose(1, 0, 2))
        ht08 = ht0.astype(FP8)

        # aut2: [128, K, 2, P]: plane 0 = action embedding rows 0-31 +
        # constant-1 bias row 32; plane 1 = zeros (DoubleRow zero k-tile)
        au_c = au_full[sl].transpose(1, 0, 2, 3).reshape(K, P, ED)  # [K,P,ED]
        aut2 = np.zeros((128, K, 2, P), np.float32)
        aut2[:ED, :, 0, :] = au_c.transpose(2, 0, 1)
        aut2[ED, :, 0, :] = 1.0
        aut2 = aut2.astype(FP8)

        # gi_n: n-gate input contribution (+ bih n-part), host-precomputed,
        # k-major in DRAM so per-step slices are independent DMAs:
        # gin[k, p, ct, j] = (au_c[k, j] @ wih_n.T + bih_n)[ct*128 + p]
        gi = au_c @ wih[2 * H:].T + bih[2 * H:]                     # [K,P,H]
        gin = np.ascontiguousarray(
            gi.transpose(0, 2, 1).reshape(K, HKC, 128, P)
            .transpose(0, 2, 1, 3)).astype(BF16)                   # [K,128,4,P]

        # ftt: [128, 4, PF] in consumption (fi) order
        ft_c = ft_full[sl][:, forder].transpose(3, 1, 0, 2).reshape(H, PF)
        ftt = np.ascontiguousarray(
            ft_c.reshape(HKC, 128, PF).transpose(1, 0, 2)).astype(FP8)

        # masks, fi-ordered position flat index = fi*P + i*TS + s;
        # stored in the per-fi 128-row tail layout
        posflat = np.ascontiguousarray(
            maskf[sl][:, forder].transpose(1, 0, 2)).reshape(PF)    # [768]
        negflat = np.repeat(posflat, NNEG)                          # [15360]
        mskp = np.ascontiguousarray(
            posflat.reshape(FS, 128, PH // 128).transpose(1, 0, 2)
            .reshape(128, PF // 128)).astype(BF16)
        mskn = np.ascontiguousarray(
            negflat.reshape(FS, 128, NH // 128).transpose(1, 0, 2)
            .reshape(128, NSLOT // 128)).astype(BF16)
        denoms.append(float(posflat.sum()))

        # negatives: host gather + transpose, chunk-major fp8
        # negt[p, m, i, s] = pool8[v[m*SC+s], i*128+p]
        v = np.concatenate([negi4[f, sl].reshape(-1) for f in forder])
        g = pool8[v]                                                # [NSLOT,512]
        negt = np.ascontiguousarray(
            g.reshape(NSC, SC, HKC, 128).transpose(3, 0, 2, 1))     # [128,NSC,4,SC]

        in_maps.append(dict(
            whhc=whhc, aut2=np.ascontiguousarray(aut2),
            ht08=ht08,
            w1ap=w1ap, w1bp=w1bp, w2p=w2p, w3p=w3p,
            b1t=b1_dev, b2t=b2_dev, b3c=b3c,
            ftt=ftt, negt=negt, mskn=mskn, mskp=mskp,
            gin=gin,
        ))

    return in_maps, tuple(int(u) for u in usub), sum(denoms)


# ----------------------------------------------------------------------------
# device program
# ----------------------------------------------------------------------------

def _build(usub_vals):
    import concourse.bass as bass
    import concourse.bacc as bacc
    import concourse.mybir as mybir
    import concourse.tile as tile

    dt = mybir.dt
    AF = mybir.ActivationFunctionType
    AL = mybir.AluOpType
    DR = mybir.MatmulPerfMode.DoubleRow
    RELU_ADD = _relu_add_op()

    forder = sorted(range(FS), key=lambda f: (usub_vals[f], f))
    parkn = min(NSC, 23)

    nc = bacc.Bacc("TRN2", target_bir_lowering=False, debug=False,
                   num_devices=NCORE)

    def din(name, shape, d):
        return nc.dram_tensor(name, shape, d, kind="ExternalInput").ap()

    whhc = din("whhc", [128, 3, 2, 3 * H], dt.float8e4)
    aut2 = din("aut2", [128, K, 2, P], dt.float8e4)
    ht08 = din("ht08", [128, HKC, P], dt.float8e4)
    w1ap = din("w1ap", [128, 2, 2, H], dt.float8e4)
    w1bp = din("w1bp", [128, 2, 2, H], dt.float8e4)
    w2p = din("w2p", [128, 2, 2, H], dt.float8e4)
    w3p = din("w3p", [128, 2, 2, 128], dt.float8e4)
    b1t = din("b1t", [128, HKC], dt.float32)
    b2t = din("b2t", [128, HKC], dt.float32)
    b3c = din("b3c", [128, 2], dt.float32)
    ftt = din("ftt", [128, HKC, PF], dt.float8e4)
    gind = din("gin", [K, 128, HKC, P], dt.bfloat16)
    negd = din("negt", [128, NSC, HKC, SC], dt.float8e4)
    msknd = din("mskn", [128, NSLOT // 128], dt.bfloat16)
    mskpd = din("mskp", [128, PF // 128], dt.bfloat16)
    out = nc.dram_tensor("out", [1, 4], dt.float32, kind="ExternalOutput").ap()

    with tile.TileContext(nc) as tc:
        with (
            tc.tile_pool(name="cw", bufs=1) as cw,
            tc.tile_pool(name="ps2", bufs=6, space="PSUM") as ps2,
            tc.tile_pool(name="plp", bufs=2, space="PSUM") as plp,
            tc.tile_pool(name="ng", bufs=5) as ng,
            tc.tile_pool(name="lr", bufs=4) as lr,
            tc.tile_pool(name="dsc", bufs=1, space="DRAM") as dsc,
        ):
            # logit rows land here via a small SBUF staging row (bf16)
            dROW = dsc.tile([FS, NH + PH], dt.bfloat16, name="drow")

            # ---------------- DMA: priority order ----------------
            # group A: GRU-critical (scan starts as soon as these land)
            tWHH = cw.tile([128, 3, 2, 3 * H], dt.float8e4, tag="whhc",
                           name="whhc")
            tC8 = [cw.tile([128, HKC, P], dt.float8e4, tag=f"c8{i}",
                           name=f"c8{i}") for i in range(2)]
            tAUT = cw.tile([128, K, 2, P], dt.float8e4, tag="aut2",
                           name="aut2")
            tGIN = cw.tile([128, K, HKC, P], dt.bfloat16, tag="gin",
                           name="gin")
            nc.sync.dma_start(out=tWHH[:, 0], in_=whhc[:, 0])
            nc.sync.dma_start(out=tC8[0][:], in_=ht08[:])
            nc.sync.dma_start(out=tWHH[:, 2], in_=whhc[:, 2])
            nc.sync.dma_start(out=tAUT[:, 0:4], in_=aut2[:, 0:4])
            nc.sync.dma_start(out=tWHH[:, 1], in_=whhc[:, 1])
            nc.sync.dma_start(out=tGIN[:, 0], in_=gind[0])
            nc.sync.dma_start(out=tAUT[:, 4:], in_=aut2[:, 4:])

            # group B: streamed in consumption order (in-order DMA queue)
            tNEG = cw.tile([128, NSC, HKC, SC], dt.float8e4, tag="negt",
                           name="negt")
            tW1A = cw.tile([128, 2, 2, H], dt.float8e4, tag="w1ap", name="w1ap")
            tW1B = cw.tile([128, 2, 2, H], dt.float8e4, tag="w1bp", name="w1bp")
            tW2 = cw.tile([128, 2, 2, H], dt.float8e4, tag="w2p", name="w2p")
            tW3 = cw.tile([128, 2, 2, 128], dt.float8e4, tag="w3p", name="w3p")
            tB1 = cw.tile([128, HKC], dt.float32, tag="b1t", name="b1t")
            tB2 = cw.tile([128, HKC], dt.float32, tag="b2t", name="b2t")
            tB3C = cw.tile([128, 2], dt.float32, tag="b3c", name="b3c")
            tFTT = cw.tile([128, HKC, PF], dt.float8e4, tag="ftt", name="ftt")
            tMSKN = cw.tile([128, NSLOT // 128], dt.bfloat16, tag="mskn",
                            name="mskn")
            tMSKP = cw.tile([128, PF // 128], dt.bfloat16, tag="mskp",
                            name="mskp")

            nc.sync.dma_start(out=tNEG[:, 0:2], in_=negd[:, 0:2])
            nc.sync.dma_start(out=tGIN[:, 1], in_=gind[1])
            nc.sync.dma_start(out=tNEG[:, 2:4], in_=negd[:, 2:4])
            nc.sync.dma_start(out=tGIN[:, 2], in_=gind[2])
            nc.sync.dma_start(out=tW1A[:], in_=w1ap[:])
            nc.sync.dma_start(out=tW1B[:], in_=w1bp[:])
            nc.sync.dma_start(out=tB1[:], in_=b1t[:])
            nc.sync.dma_start(out=tNEG[:, 4:6], in_=negd[:, 4:6])
            nc.sync.dma_start(out=tGIN[:, 3], in_=gind[3])
            nc.sync.dma_start(out=tNEG[:, 6:8], in_=negd[:, 6:8])
            nc.sync.dma_start(out=tGIN[:, 4], in_=gind[4])
            nc.sync.dma_start(out=tW2[:], in_=w2p[:])
            nc.sync.dma_start(out=tW3[:], in_=w3p[:])
            nc.sync.dma_start(out=tB2[:], in_=b2t[:])
            nc.sync.dma_start(out=tB3C[:], in_=b3c[:])
            nc.sync.dma_start(out=tNEG[:, 8:10], in_=negd[:, 8:10])
            nc.sync.dma_start(out=tGIN[:, 5], in_=gind[5])
            nc.sync.dma_start(out=tNEG[:, 10:12], in_=negd[:, 10:12])
            nc.sync.dma_start(out=tGIN[:, 6], in_=gind[6])
            nc.sync.dma_start(out=tFTT[:], in_=ftt[:])
            nc.sync.dma_start(out=tGIN[:, 7], in_=gind[7])
            nc.sync.dma_start(out=tNEG[:, 12:16], in_=negd[:, 12:16])
            nc.sync.dma_start(out=tNEG[:, 16:20], in_=negd[:, 16:20])
            nc.sync.dma_start(out=tNEG[:, 20:24], in_=negd[:, 20:24])
            nc.sync.dma_start(out=tNEG[:, 24:28], in_=negd[:, 24:28])
            nc.sync.dma_start(out=tNEG[:, 28:32], in_=negd[:, 28:32])
            nc.sync.dma_start(out=tMSKN[:], in_=msknd[:])
            nc.sync.dma_start(out=tMSKP[:], in_=mskpd[:])

            # persistent state tiles
            tAT = cw.tile([128, HKC, PF], dt.bfloat16, tag="at", name="at")
            tR = cw.tile([128, HKC, P], dt.bfloat16, tag="r", name="r")
            tZ = cw.tile([128, HKC, P], dt.bfloat16, tag="z", name="z")
            tLV = cw.tile([128, NSLOT // 128], dt.bfloat16, tag="lv", name="lv")
            tLPV = cw.tile([128, PF // 128], dt.bfloat16, tag="lpv", name="lpv")
            tAN = cw.tile([128, 8], dt.float32, tag="an", name="an")
            tONE = cw.tile([128, 1], dt.float32, tag="one", name="one")
            nc.vector.memset(tONE[:], 1.0)
            tRES = cw.tile([1, 4], dt.float32, tag="res", name="res")

            # ---------------- negative-chunk helpers ----------------
            tPARK = cw.tile([128, max(parkn, 1), HKC, SC], dt.float8e4,
                            tag="park", name="park")

            def l1_matmuls(m, dst4):
                """W1b @ negt chunk m into four 1-bank PSUM tiles."""
                for ht in range(HKC):
                    p1 = ps2.tile([128, 512], dt.float32, tag="ps")
                    for g in range(2):
                        nc.tensor.matmul(
                            p1[:, :SC],
                            lhsT=tW1B[:, g, :, ht * 128:(ht + 1) * 128],
                            rhs=tNEG[:, m, 2 * g:2 * g + 2, :],
                            start=(g == 0), stop=(g == 1), perf_mode=DR)
                    dst4[ht] = p1

            # ---- per-fi tail: DRAM bounces per piece (8/4/4 chunks); the
            # softplus block runs in two stages so only the tiny last piece
            # is serialized after the final chunk ----
            POFF = (0, NH // 2, 3 * NH // 4, NH)

            def bounce_piece(o, pc):
                c0, c1 = POFF[pc], POFF[pc + 1]
                nc.sync.dma_start(
                    out=tLV[:, (o * NH + c0) // 128:(o * NH + c1) // 128],
                    in_=dROW[o:o + 1, c0:c1]
                    .rearrange("a (p c) -> (a p) c", p=128))

            def bounce_pos(o):
                pcol = PH // 128
                nc.sync.dma_start(
                    out=tLPV[:, o * pcol:(o + 1) * pcol],
                    in_=dROW[o:o + 1, NH:]
                    .rearrange("a (p c) -> (a p) c", p=128))

            def tail_stage1():
                # everything except the last piece of the last fi; exps
                # grouped before lns so each act table loads exactly once
                w = (NSLOT - NH // 4) // 128
                nc.scalar.activation(out=tLV[:, :w], in_=tLV[:, :w],
                                     func=AF.Exp, bias=tB3C[:, 0:1])
                nc.scalar.activation(out=tLPV[:], in_=tLPV[:], func=AF.Exp,
                                     scale=-1.0, bias=tB3C[:, 1:2])
                nc.vector.tensor_mul(out=tLV[:, :w], in0=tLV[:, :w],
                                     in1=tMSKN[:, :w])
                nc.vector.tensor_mul(out=tLPV[:], in0=tLPV[:], in1=tMSKP[:])
                nc.scalar.activation(out=tLV[:, :w], in_=tLV[:, :w],
                                     func=AF.Ln, bias=1.0,
                                     accum_out=tAN[:, 1:2])
                nc.scalar.activation(out=tLPV[:], in_=tLPV[:], func=AF.Ln,
                                     bias=1.0, accum_out=tAN[:, 0:1])

            def tail_stage2():
                # rows already hold exp(x+b3); only mul + ln remain and the
                # ln table is already loaded
                w = (NSLOT - NH // 4) // 128
                nc.vector.tensor_mul(out=tLV[:, w:], in0=tLV[:, w:],
                                     in1=tMSKN[:, w:])
                nc.scalar.activation(out=tLV[:, w:], in_=tLV[:, w:],
                                     func=AF.Ln, bias=1.0,
                                     accum_out=tAN[:, 2:3])
                nc.vector.tensor_add(out=tAN[:, 1:2], in0=tAN[:, 1:2],
                                     in1=tAN[:, 2:3])
                for col, srcc in ((0, 0), (1, 1)):
                    pr = plp.tile([128, 512], dt.float32, tag="pl",
                                  name="pr")
                    nc.tensor.matmul(pr[:1, :1],
                                     lhsT=tAN[:, srcc:srcc + 1],
                                     rhs=tONE[:], start=True, stop=True)
                    nc.vector.tensor_copy(out=tRES[0:1, col:col + 1],
                                          in_=pr[:1, :1])
                nc.vector.memset(tRES[0:1, 2:4], 0.0)
                nc.sync.dma_start(out=out[:], in_=tRES[:])

            # ---------------- per-f section (generator) ----------------
            # fi below is the consumption-order position (0 = first ready);
            # chunk indices, tAT/ftt/mask columns all use this ordering.
            def emit_f_section(fi, n8):
                cols = slice(fi * P, (fi + 1) * P)
                # AT = W1a @ fp + b1  (fp = n8) — MUST run at its step (n8
                # is recycled two steps later)
                for ht in range(HKC):
                    p1 = ps2.tile([128, 512], dt.float32, tag="ps")
                    for g in range(2):
                        nc.tensor.matmul(
                            p1[:, :P],
                            lhsT=tW1A[:, g, :, ht * 128:(ht + 1) * 128],
                            rhs=n8[:, 2 * g:2 * g + 2, :],
                            start=(g == 0), stop=(g == 1), perf_mode=DR)
                    nc.scalar.activation(
                        out=tAT[:, ht, cols], in_=p1[:, :P],
                        func=AF.Identity, bias=tB1[:, ht:ht + 1])
                yield
                # positives: h1 = relu(W1b@ft + AT); h2 = relu(W2@h1+b2)
                h1 = ng.tile([128, HKC, P], dt.float8e4, tag="h1", name="h1p")
                for ht in range(HKC):
                    p1 = ps2.tile([128, 512], dt.float32, tag="ps")
                    for g in range(2):
                        nc.tensor.matmul(
                            p1[:, :P],
                            lhsT=tW1B[:, g, :, ht * 128:(ht + 1) * 128],
                            rhs=tFTT[:, 2 * g:2 * g + 2, cols],
                            start=(g == 0), stop=(g == 1), perf_mode=DR)
                    nc.vector._custom_dve(
                        RELU_ADD, out=h1[:, ht, :], in0=p1[:, :P],
                        in1=tAT[:, ht, cols])
                yield
                h2 = ng.tile([128, HKC, P], dt.float8e4, tag="h2", name="h2p")
                for ht in range(HKC):
                    p1 = ps2.tile([128, 512], dt.float32, tag="ps")
                    for g in range(2):
                        nc.tensor.matmul(
                            p1[:, :P],
                            lhsT=tW2[:, g, :, ht * 128:(ht + 1) * 128],
                            rhs=h1[:, 2 * g:2 * g + 2, :],
                            start=(g == 0), stop=(g == 1), perf_mode=DR)
                    nc.scalar.activation(
                        out=h2[:, ht, :], in_=p1[:, :P],
                        func=AF.Relu, bias=tB2[:, ht:ht + 1])
                pl = plp.tile([128, 512], dt.float32, tag="pl")
                for g in range(2):
                    nc.tensor.matmul(
                        pl[:, :P], lhsT=tW3[:, g],
                        rhs=h2[:, 2 * g:2 * g + 2, :],
                        start=(g == 0), stop=(g == 1), perf_mode=DR)
                row = lr.tile([1, SC], dt.bfloat16, tag="lrow", name="rowp")
                nc.scalar.activation(out=row[0:1, :P], in_=pl[0:1, :P],
                                     func=AF.Identity)
                nc.sync.dma_start(out=dROW[fi:fi + 1, NH:],
                                  in_=row[0:1, :P])
                bounce_pos(fi)
                yield
                # negatives, software-pipelined: L1(m) | L2(m-1) | L3(m-2)
                h1s, h2s = {}, {}
                for m in range(SCF + 2):
                    if m < SCF:
                        cm = fi * SCF + m
                        a0 = fi * P + m * 24
                        h1n = ng.tile([128, HKC, SC], dt.float8e4, tag="h1",
                                      name="h1n")
                        if cm < parkn:
                            for ht in range(HKC):
                                nc.vector._custom_dve(
                                    RELU_ADD,
                                    out=h1n[:, ht, :].rearrange(
                                        "p (a b) -> p a b", b=NNEG),
                                    in0=tPARK[:, cm, ht, :].rearrange(
                                        "p (a b) -> p a b", b=NNEG),
                                    in1=tAT[:, ht, a0:a0 + 24][:, :, None]
                                    .broadcast_to((128, 24, NNEG)))
                        else:
                            p4 = [None] * HKC
                            l1_matmuls(cm, p4)
                            for ht in range(HKC):
                                nc.vector._custom_dve(
                                    RELU_ADD,
                                    out=h1n[:, ht, :].rearrange(
                                        "p (a b) -> p a b", b=NNEG),
                                    in0=p4[ht][:, :SC].rearrange(
                                        "p (a b) -> p a b", b=NNEG),
                                    in1=tAT[:, ht, a0:a0 + 24][:, :, None]
                                    .broadcast_to((128, 24, NNEG)))
                        h1s[m] = h1n
                    if 1 <= m <= SCF:
                        h1n = h1s.pop(m - 1)
                        h2n = ng.tile([128, HKC, SC], dt.float8e4, tag="h2",
                                      name="h2n")
                        for ht in range(HKC):
                            p1 = ps2.tile([128, 512], dt.float32, tag="ps")
                            for g in range(2):
                                nc.tensor.matmul(
                                    p1[:, :SC],
                                    lhsT=tW2[:, g, :,
                                             ht * 128:(ht + 1) * 128],
                                    rhs=h1n[:, 2 * g:2 * g + 2, :],
                                    start=(g == 0), stop=(g == 1),
                                    perf_mode=DR)
                            nc.scalar.activation(
                                out=h2n[:, ht, :], in_=p1[:, :SC],
                                func=AF.Relu, bias=tB2[:, ht:ht + 1])
                        h2s[m - 1] = h2n
                    if m >= 2:
                        ml = m - 2
                        h2n = h2s.pop(ml)
                        pl = plp.tile([128, 512], dt.float32, tag="pl")
                        for g in range(2):
                            nc.tensor.matmul(
                                pl[:, :SC], lhsT=tW3[:, g],
                                rhs=h2n[:, 2 * g:2 * g + 2, :],
                                start=(g == 0), stop=(g == 1), perf_mode=DR)
                        row = lr.tile([1, SC], dt.bfloat16, tag="lrow",
                                      name="rown")
                        if fi == FS - 1 and ml >= 3 * SCF // 4:
                            # last piece: store exp(x+b3) so the final stage
                            # needs only the ln table (no extra table swap)
                            nc.scalar.activation(out=row[:], in_=pl[0:1, :SC],
                                                 func=AF.Exp,
                                                 bias=tB3C[0:1, 0:1])
                        elif ml % 2 == 0:
                            nc.vector.tensor_copy(out=row[:], in_=pl[0:1, :SC])
                        else:
                            nc.scalar.activation(out=row[:], in_=pl[0:1, :SC],
                                                 func=AF.Identity)
                        nc.sync.dma_start(
                            out=dROW[fi:fi + 1, ml * SC:(ml + 1) * SC],
                            in_=row[:])
                    if m - 2 == SCF // 2 - 1:
                        bounce_piece(fi, 0)
                    elif m - 2 == 3 * SCF // 4 - 1:
                        bounce_piece(fi, 1)
                    if fi == FS - 1 and m == SCF + 1:
                        # Scalar is past its last relu; exp/ln are grouped so
                        # each table loads once, hidden inside the drain
                        tail_stage1()
                    yield
                bounce_piece(fi, 2)
                if fi == FS - 1:
                    tail_stage2()

            def park_chunk(m):
                """L1 matmuls for chunk m + pre-activation parked to SBUF
                fp8 (copies split DVE/Scalar; AT-add applied post-scan).
                Bridges the per-step chain-tail PE gap so the PE pstate
                stays ramped."""
                p4 = [None] * HKC
                l1_matmuls(m, p4)
                for ht in range(HKC):
                    eng = nc.vector.tensor_copy if ht % 2 == 0 else None
                    if eng:
                        eng(out=tPARK[:, m, ht, :], in_=p4[ht][:, :SC])
                    else:
                        nc.scalar.activation(out=tPARK[:, m, ht, :],
                                             in_=p4[ht][:, :SC],
                                             func=AF.Identity)

            # ---------------- GRU scan + interleaving ----------------
            parked = [0]
            pending = []
            for k in range(K):
                c8, n8 = tC8[k % 2], tC8[(k + 1) % 2]

                # Each matmul group is split: kt01+aut passes depend only on
                # the previous step's cp0 state half, kt23 on cp1.  Emitting
                # all cp0-dependent passes first lets the PE bridge the
                # previous step's chain tail.
                def open_grp(gts, tag):
                    ps = []
                    for gt in gts:
                        p1 = ps2.tile([128, 512], dt.float32, tag="ps")
                        nc.tensor.matmul(
                            p1[:, :P],
                            lhsT=tWHH[:, 0, :, gt * 128:(gt + 1) * 128],
                            rhs=c8[:, 0:2, :],
                            start=True, stop=False, perf_mode=DR)
                        nc.tensor.matmul(
                            p1[:, :P],
                            lhsT=tWHH[:, 2, :, gt * 128:(gt + 1) * 128],
                            rhs=tAUT[:, k],
                            start=False, stop=False, perf_mode=DR)
                        ps.append(p1)
                    return ps

                def close_grp(ps, gts):
                    for p1, gt in zip(ps, gts):
                        nc.tensor.matmul(
                            p1[:, :P],
                            lhsT=tWHH[:, 1, :, gt * 128:(gt + 1) * 128],
                            rhs=c8[:, 2:4, :],
                            start=False, stop=True, perf_mode=DR)

                def sig(ps, dst, cp, split=False):
                    for j in range(2):
                        nc.scalar.activation(
                            out=dst[:, cp + j, :], in_=ps[j][:, :P],
                            func=AF.Sigmoid)

                def chain_a(cp, phs):
                    # per-j (single ct tile) ops halve each link's latency
                    # on the serial path; j1 trails j0 by one engine slot
                    t2 = ng.tile([128, 2, P], dt.bfloat16, tag="tm", name="t2")
                    c2 = ng.tile([128, 2, P], dt.bfloat16, tag="tm", name="c2")
                    d2 = ng.tile([128, 2, P], dt.bfloat16, tag="tm", name="d2")
                    for j in range(2):
                        ct = cp * 2 + j
                        # psum = gh_n + bhh_n (bias row in the aut pass), so
                        # the r-mult is a plain tensor_tensor
                        nc.vector.tensor_mul(out=t2[:, j, :],
                                             in0=phs[j][:, :P],
                                             in1=tR[:, ct, :])
                        nc.vector.tensor_add(out=t2[:, j, :], in0=t2[:, j, :],
                                             in1=tGIN[:, k, ct, :])
                        nc.scalar.activation(out=c2[:, j, :], in_=t2[:, j, :],
                                             func=AF.Tanh)
                        nc.vector.tensor_sub(out=d2[:, j, :],
                                             in0=c8[:, ct, :],
                                             in1=c2[:, j, :])
                    return c2, d2

                def chain_b(cp, c2, d2):
                    for j in range(2):
                        ct = cp * 2 + j
                        nc.vector.tensor_mul(out=d2[:, j, :], in0=d2[:, j, :],
                                             in1=tZ[:, ct, :])
                        # fp8 state: the final add writes the next-step state
                        # tile directly (no separate bf16 state / cast)
                        nc.vector.tensor_add(out=n8[:, ct, :],
                                             in0=d2[:, j, :],
                                             in1=c2[:, j, :])

                if k >= 3 and parked[0] < parkn:
                    park_chunk(parked[0]); parked[0] += 1
                # phase 1: cp0-dependent partials for r01, r23, n0
                pA = open_grp((0, 1), "r01")
                pB = open_grp((2, 3), "r23")
                pC = open_grp((8, 9), "n0")
                # phase 2: cp1-dependent closes; n0 closes right after the
                # r01 sigmoid so the cp0 chain launches ASAP
                close_grp(pA, (0, 1))
                sig(pA, tR, 0, split=True)
                close_grp(pC, (8, 9))
                ca0 = chain_a(0, pC)
                close_grp(pB, (2, 3))
                sig(pB, tR, 2, split=True)
                # n1 full group
                pD = []
                for gt in (10, 11):
                    p1 = ps2.tile([128, 512], dt.float32, tag="ps")
                    nc.tensor.matmul(
                        p1[:, :P],
                        lhsT=tWHH[:, 0, :, gt * 128:(gt + 1) * 128],
                        rhs=c8[:, 0:2, :],
                        start=True, stop=False, perf_mode=DR)
                    nc.tensor.matmul(
                        p1[:, :P],
                        lhsT=tWHH[:, 2, :, gt * 128:(gt + 1) * 128],
                        rhs=tAUT[:, k],
                        start=False, stop=False, perf_mode=DR)
                    nc.tensor.matmul(
                        p1[:, :P],
                        lhsT=tWHH[:, 1, :, gt * 128:(gt + 1) * 128],
                        rhs=c8[:, 2:4, :],
                        start=False, stop=True, perf_mode=DR)
                    pD.append(p1)
                pE = open_grp((4, 5), "z01")
                close_grp(pE, (4, 5))
                sig(pE, tZ, 0)
                chain_b(0, *ca0)
                ca1 = chain_a(1, pD)
                pF = open_grp((6, 7), "z23")
                close_grp(pF, (6, 7))
                sig(pF, tZ, 2)
                chain_b(1, *ca1)
                                # chain-tail filler: parked L1 chunks keep the PE busy
                # (and its pstate ramped) across the serial chain tail
                if parked[0] + 2 <= parkn:
                    park_chunk(parked[0])
                    park_chunk(parked[0] + 1)
                    parked[0] += 2
                for fi in range(FS):
                    if usub_vals[forder[fi]] == k:
                        sec = emit_f_section(fi, n8)
                        next(sec)           # AT stage now; rest post-scan
                        pending.append(sec)
                if k == K - 1:
                    # bridge the scan->drain transition (the k=7 chain tail)
                    # with the last two parked chunks
                    while parked[0] < parkn:
                        park_chunk(parked[0]); parked[0] += 1
                    while pending:
                        try:
                            next(pending[0])
                        except StopIteration:
                            pending.pop(0)

    nc.compile()
    return nc


def _get_program(usub_vals):
    key = usub_vals
    if key not in _PROG_CACHE:
        _PROG_CACHE[key] = _build(usub_vals)
    return _PROG_CACHE[key]


def kernel(**inputs):
    from concourse.bass_utils import run_bass_kernel_spmd
    in_maps, usub_vals, denom = _prep(inputs)
    nc = _get_program(usub_vals)
    res = run_bass_kernel_spmd(nc, in_maps, list(range(NCORE)))
    parts = np.stack([np.asarray(res.results[c]['out'][0], np.float64)
                      for c in range(NCORE)])
    pos, neg = parts[:, 0].sum(), parts[:, 1].sum()
    return np.float32(0.1 * (pos / denom + neg / (denom * NNEG)))


# revision 44
# speedup vs baseline: 1.0544x; 1.0429x over previous
"""Trainium2 Bass kernel for nn_CPCA (CPC-action loss).

Strategy: data-parallel over the env dim n (64 envs/core on 8 cores).
v4 (from the v3 indirect-gather baseline, 319us -> ~239us):
  - negatives are gathered AND transposed on the host into a chunk-major
    fp8 tensor streamed to SBUF by plain DMA.  This removes v3's 120
    serialized indirect DMAs (~1.1us each), 240 PE transposes and 120
    PSUM->SBUF copies that rate-limited the first 120us.
  - DMA issue order prioritized: GRU-critical tensors first (scan starts
    ~10us in), negatives/gin/weights stream in consumption order.
  - all PSUM tiles are single-bank [128,512] with a 6-deep rotation;
    the finer recycle granularity removes most PE psum-wait stalls.
  - GRU step emission is split by state-half dependency: kt01+aut
    partial-accumulations (needing only the previous step's first
    state half) issue before the kt23 closes, so the PE bridges the
    serial chain tail; r-gates and n-gates go first, z-gates ride in
    the chain's shadow.  The state stays in fp8 (tC8 tiles) and the
    update chain runs per-ct-tile to halve each link's latency.
  - bhh_n rides the n-matmul bias row so the r-mult is a plain wide
    tensor_tensor; gi_n (+bih_n) is host-precomputed (gin).
  - L1 matmuls for 21 of the 32 negative chunks run during the scan as
    PE filler ("parking": pre-activation copied to SBUF fp8, the
    relu(x+AT) applied post-scan) keeping the PE pstate ramped.
  - sections (AT/pos/neg pipelines) drain post-scan, software-pipelined
    L1(m)|L2(m-1)|L3(m-2); only each fi's AT stage runs at its step
    (its n8 state tile is recycled two steps later).
  - logit rows go PSUM -> small SBUF row -> DRAM, bounced back per
    8/4/4-chunk piece into a 128-row layout; softplus accumulation runs
    in two stages with exp/ln grouped so the act table loads exactly
    once per function family, overlapping the drain.
Per-core partial sums (pos_loss_sum, neg_loss_sum) are combined with
the host-side mask count into the scalar loss.
"""
import sys

if '/opt/trn_rl_repo' not in sys.path:
    sys.path.insert(0, '/opt/trn_rl_repo')

import numpy as np
import ml_dtypes

BF16 = ml_dtypes.bfloat16
FP8 = ml_dtypes.float8_e4m3   # IEEE e4m3 (max 240) == TRN fp8_exp4

N, T, H, TS, FS, K, A, ED, NNEG = 512, 128, 512, 6, 2, 8, 17, 32, 20
NCORE = 8
NE = N // NCORE          # 64 envs per core
P = NE * TS              # 384 positions per core (per unroll index)
PF = FS * P              # 768
NSLOT = FS * P * NNEG    # 15360 negative slots per core
SC = 24 * NNEG           # 480 slots (24 positions) per matmul sub-chunk
NSC = NSLOT // SC        # 32 sub-chunks (16 per unroll index)
SCF = NSC // FS          # 16
HKC = H // 128           # 4
NH = NSLOT // FS         # 7680 neg logits per fi
PH = P                   # 384 pos logits per fi

_PROG_CACHE = {}


# ----------------------------------------------------------------------------
# custom DVE op: out = relu(in0 + in1)   (in1 may be a stride-0 broadcast)
# ----------------------------------------------------------------------------

def _relu_add_op():
    from concourse import dve_ops
    from concourse.dve_spec import Spec, Src0, Src1, relu, lower
    from concourse.dve_uop import DveOpSpec

    name = "RELU_ADD_CPCA"
    for op in dve_ops.OPS:
        if op.name == name:
            return op

    def _ref(in0, in1, c0, c1, c2):
        x = np.asarray(in0, np.float32) + \
            np.asarray(in1, np.float32).reshape(np.asarray(in0).shape)
        return np.maximum(
            np.nan_to_num(x, nan=0.0, posinf=np.inf, neginf=-np.inf), 0)

    spec = Spec(body=relu(Src0 + Src1), reference=_ref)
    shas = {}
    for ver in ("v3", "v4"):
        tmp = DveOpSpec(name=name, opcode=31, uops=lower(spec, ver=ver),
                        rd1_en=True)
        shas[ver] = tmp.sha(ver)
    op = dve_ops.DveOp(name, spec, subdim=False, uops_sha=shas)
    dve_ops.OPS.append(op)
    dve_ops.CUSTOM_DVE_SPECS[name] = spec
    dve_ops._SUB_OPCODE_FOR_NAME[name] = (
        dve_ops._CUSTOM_DVE_ROW_BASE + len(dve_ops.OPS) - 1)
    assert dve_ops._SUB_OPCODE_FOR_NAME[name] < 0x20
    return op


# ----------------------------------------------------------------------------
# host-side input preparation (sharding / layout / index metadata only)
# ----------------------------------------------------------------------------

def _prep(inputs):
    acts = np.asarray(inputs['actions']).astype(np.int64)          # [N,T,1]
    nd = np.asarray(inputs['not_dones'], np.float32)               # [N,T,1]
    vld = np.asarray(inputs['valids']).astype(bool)                # [N,T,1]
    ri = np.asarray(inputs['rnn_inputs'], np.float32)              # [N,T,H]
    ro = np.asarray(inputs['rnn_outputs'], np.float32)             # [N,T,H]
    embw = np.asarray(inputs['embed_w'], np.float32)               # [A,ED]
    wih = np.asarray(inputs['gru_w_ih'], np.float32)               # [3H,ED]
    whh = np.asarray(inputs['gru_w_hh'], np.float32)               # [3H,H]
    bih = np.asarray(inputs['gru_b_ih'], np.float32)               # [3H]
    bhh = np.asarray(inputs['gru_b_hh'], np.float32)               # [3H]
    w1 = np.asarray(inputs['p_w1'], np.float32)                    # [H,2H]
    b1 = np.asarray(inputs['p_b1'], np.float32)                    # [H]
    w2 = np.asarray(inputs['p_w2'], np.float32)                    # [H,H]
    b2 = np.asarray(inputs['p_b2'], np.float32)                    # [H]
    w3 = np.asarray(inputs['p_w3'], np.float32)                    # [1,H]
    b3 = np.asarray(inputs['p_b3'], np.float32)                    # [1]
    tsub = np.asarray(inputs['time_subsample']).astype(np.int64)   # [TS]
    usub = np.asarray(inputs['unroll_subsample']).astype(np.int64) # [FS]
    negi = np.asarray(inputs['neg_indices']).astype(np.int64)      # [FS*TS*N*NNEG]
    maxk = int(np.asarray(inputs['max_k']))
    assert maxk == K, maxk
    assert tsub.shape == (TS,) and usub.shape == (FS,)

    forder = np.argsort(usub, kind='stable')                       # consumption order

    # ---- shared (replicated) tensors -------------------------------------
    def dr_std(w):
        # [p, g, i, m] = w[m, (2g+i)*128 + p]
        return np.ascontiguousarray(
            w.T.reshape(2, 2, 128, -1).transpose(2, 0, 1, 3)).astype(FP8)

    # GRU combined lhsT: k-tiles 0-3 = whh.T chunks, 4 = wih.T (+bias row 32),
    # 5 = zeros.  Gates r/z get bih+bhh via the bias row; the n-gate columns
    # of the aut plane carry ONLY bhh_n in the bias row (wih_n zeroed - the
    # ih part arrives host-precomputed via gin), so the n psum is directly
    # gh_n + bhh_n and the r-mult is a plain tensor_tensor multiply.
    wk = np.zeros((6, 128, 3 * H), np.float32)
    wk[:4] = whh.T.reshape(4, 128, 3 * H)
    wk[4, :ED, :2 * H] = wih.T[:, :2 * H]
    wk[4, ED] = np.concatenate([(bih + bhh)[:2 * H], bhh[2 * H:]])
    whhc = np.ascontiguousarray(
        wk.reshape(3, 2, 128, 3 * H).transpose(2, 0, 1, 3)).astype(FP8)

    w1ap = dr_std(w1[:, :H])
    w1bp = dr_std(w1[:, H:])
    w2p = dr_std(w2)
    # layer-3 weights broadcast to all 128 output rows — skinny (M<4) DR
    # ldweights fail the walrus ISA check; only PSUM partition 0 is read.
    w3p = np.ascontiguousarray(np.broadcast_to(
        w3[0].reshape(2, 2, 128).transpose(2, 0, 1)[..., None],
        (128, 2, 2, 128))).astype(FP8)

    b1_dev = np.ascontiguousarray(b1.reshape(HKC, 128).T)
    b2_dev = np.ascontiguousarray(b2.reshape(HKC, 128).T)
    b3c = np.broadcast_to(np.array([b3[0], -b3[0]], np.float32), (128, 2)).copy()

    # negatives pool in fp8 (same values the v3 device path consumed)
    pool8 = ri.reshape(N * T, H).astype(FP8)

    # ---- per-core views ---------------------------------------------------
    ks = np.arange(K)
    tq = tsub[None, :] + ks[:, None]                                # [K,TS]
    ok_au = tq <= T - 2
    a_idx = acts[:, np.clip(tq, 0, T - 1), 0]                       # [N,K,TS]
    au_full = embw[a_idx] * ok_au[None, :, :, None]                 # [N,K,TS,ED]

    tf = tsub[None, :] + usub[:, None]                              # [FS,TS]
    ok_ft = tf <= T - 2
    ft_full = np.where(ok_ft[None, :, :, None],
                       ri[:, np.clip(tf + 1, 0, T - 1)], 0.0)       # [N,FS,TS,H]

    vm = ((nd[:, :, 0] > 0) & vld[:, :, 0]).astype(np.float32)      # [N,T]
    vmk = np.where(ok_au[None], vm[:, np.clip(tq, 0, T - 1)], 0.0)  # [N,K,TS]
    cum = np.cumprod(vmk, axis=1)                                   # [N,K,TS]
    maskf = cum[:, usub, :]                                         # [N,FS,TS]

    negi4 = negi.reshape(FS, N, TS, NNEG)

    in_maps = []
    denoms = []
    for c in range(NCORE):
        sl = slice(c * NE, (c + 1) * NE)

        # h0: [128, 4, P] dev[p,kc,j] = ro[i, ts_s, kc*128+p], j = i*TS+s
        h0 = ro[sl][:, tsub].reshape(P, H).T                        # [H,P]
        ht0 = np.ascontiguousarray(h0.reshape(HKC, 128, P).transpose(1, 0, 2))
        ht08 = ht0.astype(FP8)

        # aut2: [128, K, 2, P]: plane 0 = action embedding rows 0-31 +
        # constant-1 bias row 32; plane 1 = zeros (DoubleRow zero k-tile)
        au_c = au_full[sl].transpose(1, 0, 2, 3).reshape(K, P, ED)  # [K,P,ED]
        aut2 = np.zeros((128, K, 2, P), np.float32)
        aut2[:ED, :, 0, :] = au_c.transpose(2, 0, 1)
        aut2[ED, :, 0, :] = 1.0
        aut2 = aut2.astype(FP8)

        # gi_n: n-gate input contribution (+ bih n-part), host-precomputed,
        # k-major in DRAM so per-step slices are independent DMAs:
        # gin[k, p, ct, j] = (au_c[k, j] @ wih_n.T + bih_n)[ct*128 + p]
        gi = au_c @ wih[2 * H:].T + bih[2 * H:]                     # [K,P,H]
        gin = np.ascontiguousarray(
            gi.transpose(0, 2, 1).reshape(K, HKC, 128, P)
            .transpose(0, 2, 1, 3)).astype(BF16)                   # [K,128,4,P]

        # ftt: [128, 4, PF] in consumption (fi) order
        ft_c = ft_full[sl][:, forder].transpose(3, 1, 0, 2).reshape(H, PF)
        ftt = np.ascontiguousarray(
            ft_c.reshape(HKC, 128, PF).transpose(1, 0, 2)).astype(FP8)

        # masks, fi-ordered position flat index = fi*P + i*TS + s;
        # stored in the per-fi 128-row tail layout
        posflat = np.ascontiguousarray(
            maskf[sl][:, forder].transpose(1, 0, 2)).reshape(PF)    # [768]
        negflat = np.repeat(posflat, NNEG)                          # [15360]
        mskp = np.ascontiguousarray(
            posflat.reshape(FS, 128, PH // 128).transpose(1, 0, 2)
            .reshape(128, PF // 128)).astype(BF16)
        mskn = np.ascontiguousarray(
            negflat.reshape(FS, 128, NH // 128).transpose(1, 0, 2)
            .reshape(128, NSLOT // 128)).astype(BF16)
        denoms.append(float(posflat.sum()))

        # negatives: host gather + transpose, chunk-major fp8
        # negt[p, m, i, s] = pool8[v[m*SC+s], i*128+p]
        v = np.concatenate([negi4[f, sl].reshape(-1) for f in forder])
        g = pool8[v]                                                # [NSLOT,512]
        negt = np.ascontiguousarray(
            g.reshape(NSC, SC, HKC, 128).transpose(3, 0, 2, 1))     # [128,NSC,4,SC]

        in_maps.append(dict(
            whhc=whhc, aut2=np.ascontiguousarray(aut2),
            ht08=ht08,
            w1ap=w1ap, w1bp=w1bp, w2p=w2p, w3p=w3p,
            b1t=b1_dev, b2t=b2_dev, b3c=b3c,
            ftt=ftt, negt=negt, mskn=mskn, mskp=mskp,
            gin=gin,
        ))

    return in_maps, tuple(int(u) for u in usub), sum(denoms)


# ----------------------------------------------------------------------------
# device program
# ----------------------------------------------------------------------------

def _build(usub_vals):
    import concourse.bass as bass
    import concourse.bacc as bacc
    import concourse.mybir as mybir
    import concourse.tile as tile

    dt = mybir.dt
    AF = mybir.ActivationFunctionType
    AL = mybir.AluOpType
    DR = mybir.MatmulPerfMode.DoubleRow
    RELU_ADD = _relu_add_op()

    forder = sorted(range(FS), key=lambda f: (usub_vals[f], f))
    parkn = min(NSC, 21)

    nc = bacc.Bacc("TRN2", target_bir_lowering=False, debug=False,
                   num_devices=NCORE)

    def din(name, shape, d):
        return nc.dram_tensor(name, shape, d, kind="ExternalInput").ap()

    whhc = din("whhc", [128, 3, 2, 3 * H], dt.float8e4)
    aut2 = din("aut2", [128, K, 2, P], dt.float8e4)
    ht08 = din("ht08", [128, HKC, P], dt.float8e4)
    w1ap = din("w1ap", [128, 2, 2, H], dt.float8e4)
    w1bp = din("w1bp", [128, 2, 2, H], dt.float8e4)
    w2p = din("w2p", [128, 2, 2, H], dt.float8e4)
    w3p = din("w3p", [128, 2, 2, 128], dt.float8e4)
    b1t = din("b1t", [128, HKC], dt.float32)
    b2t = din("b2t", [128, HKC], dt.float32)
    b3c = din("b3c", [128, 2], dt.float32)
    ftt = din("ftt", [128, HKC, PF], dt.float8e4)
    gind = din("gin", [K, 128, HKC, P], dt.bfloat16)
    negd = din("negt", [128, NSC, HKC, SC], dt.float8e4)
    msknd = din("mskn", [128, NSLOT // 128], dt.bfloat16)
    mskpd = din("mskp", [128, PF // 128], dt.bfloat16)
    out = nc.dram_tensor("out", [1, 4], dt.float32, kind="ExternalOutput").ap()

    with tile.TileContext(nc) as tc:
        with (
            tc.tile_pool(name="cw", bufs=1) as cw,
            tc.tile_pool(name="ps2", bufs=8, space="PSUM") as ps2,
            tc.tile_pool(name="ng", bufs=5) as ng,
            tc.tile_pool(name="lr", bufs=4) as lr,
            tc.tile_pool(name="dsc", bufs=1, space="DRAM") as dsc,
        ):
            # logit rows land here via a small SBUF staging row (bf16)
            dROW = dsc.tile([FS, NH + PH], dt.bfloat16, name="drow")

            # ---------------- DMA: priority order ----------------
            # group A: GRU-critical (scan starts as soon as these land)
            tWHH = cw.tile([128, 3, 2, 3 * H], dt.float8e4, tag="whhc",
                           name="whhc")
            tC8 = [cw.tile([128, HKC, P], dt.float8e4, tag=f"c8{i}",
                           name=f"c8{i}") for i in range(2)]
            tAUT = cw.tile([128, K, 2, P], dt.float8e4, tag="aut2",
                           name="aut2")
            tGIN = cw.tile([128, K, HKC, P], dt.bfloat16, tag="gin",
                           name="gin")
            nc.sync.dma_start(out=tWHH[:, 0], in_=whhc[:, 0])
            nc.sync.dma_start(out=tC8[0][:], in_=ht08[:])
            nc.sync.dma_start(out=tWHH[:, 2], in_=whhc[:, 2])
            nc.sync.dma_start(out=tAUT[:, 0:4], in_=aut2[:, 0:4])
            nc.sync.dma_start(out=tWHH[:, 1], in_=whhc[:, 1])
            nc.sync.dma_start(out=tGIN[:, 0], in_=gind[0])
            nc.sync.dma_start(out=tAUT[:, 4:], in_=aut2[:, 4:])

            # group B: streamed in consumption order (in-order DMA queue)
            tNEG = cw.tile([128, NSC, HKC, SC], dt.float8e4, tag="negt",
                           name="negt")
            tW1A = cw.tile([128, 2, 2, H], dt.float8e4, tag="w1ap", name="w1ap")
            tW1B = cw.tile([128, 2, 2, H], dt.float8e4, tag="w1bp", name="w1bp")
            tW2 = cw.tile([128, 2, 2, H], dt.float8e4, tag="w2p", name="w2p")
            tW3 = cw.tile([128, 2, 2, 128], dt.float8e4, tag="w3p", name="w3p")
            tB1 = cw.tile([128, HKC], dt.float32, tag="b1t", name="b1t")
            tB2 = cw.tile([128, HKC], dt.float32, tag="b2t", name="b2t")
            tB3C = cw.tile([128, 2], dt.float32, tag="b3c", name="b3c")
            tFTT = cw.tile([128, HKC, PF], dt.float8e4, tag="ftt", name="ftt")
            tMSKN = cw.tile([128, NSLOT // 128], dt.bfloat16, tag="mskn",
                            name="mskn")
            tMSKP = cw.tile([128, PF // 128], dt.bfloat16, tag="mskp",
                            name="mskp")

            nc.sync.dma_start(out=tNEG[:, 0:2], in_=negd[:, 0:2])
            nc.sync.dma_start(out=tGIN[:, 1], in_=gind[1])
            nc.sync.dma_start(out=tNEG[:, 2:4], in_=negd[:, 2:4])
            nc.sync.dma_start(out=tGIN[:, 2], in_=gind[2])
            nc.sync.dma_start(out=tW1A[:], in_=w1ap[:])
            nc.sync.dma_start(out=tW1B[:], in_=w1bp[:])
            nc.sync.dma_start(out=tB1[:], in_=b1t[:])
            nc.sync.dma_start(out=tNEG[:, 4:6], in_=negd[:, 4:6])
            nc.sync.dma_start(out=tGIN[:, 3], in_=gind[3])
            nc.sync.dma_start(out=tNEG[:, 6:8], in_=negd[:, 6:8])
            nc.sync.dma_start(out=tGIN[:, 4], in_=gind[4])
            nc.sync.dma_start(out=tW2[:], in_=w2p[:])
            nc.sync.dma_start(out=tW3[:], in_=w3p[:])
            nc.sync.dma_start(out=tB2[:], in_=b2t[:])
            nc.sync.dma_start(out=tB3C[:], in_=b3c[:])
            nc.sync.dma_start(out=tNEG[:, 8:10], in_=negd[:, 8:10])
            nc.sync.dma_start(out=tGIN[:, 5], in_=gind[5])
            nc.sync.dma_start(out=tNEG[:, 10:12], in_=negd[:, 10:12])
            nc.sync.dma_start(out=tGIN[:, 6], in_=gind[6])
            nc.sync.dma_start(out=tFTT[:], in_=ftt[:])
            nc.sync.dma_start(out=tGIN[:, 7], in_=gind[7])
            nc.sync.dma_start(out=tNEG[:, 12:16], in_=negd[:, 12:16])
            nc.sync.dma_start(out=tNEG[:, 16:20], in_=negd[:, 16:20])
            nc.sync.dma_start(out=tNEG[:, 20:24], in_=negd[:, 20:24])
            nc.sync.dma_start(out=tNEG[:, 24:28], in_=negd[:, 24:28])
            nc.sync.dma_start(out=tNEG[:, 28:32], in_=negd[:, 28:32])
            nc.sync.dma_start(out=tMSKN[:], in_=msknd[:])
            nc.sync.dma_start(out=tMSKP[:], in_=mskpd[:])

            # persistent state tiles
            tAT = cw.tile([128, HKC, PF], dt.bfloat16, tag="at", name="at")
            tR = cw.tile([128, HKC, P], dt.bfloat16, tag="r", name="r")
            tZ = cw.tile([128, HKC, P], dt.bfloat16, tag="z", name="z")
            tLV = cw.tile([128, NSLOT // 128], dt.bfloat16, tag="lv", name="lv")
            tLPV = cw.tile([128, PF // 128], dt.bfloat16, tag="lpv", name="lpv")
            tAN = cw.tile([128, 8], dt.float32, tag="an", name="an")
            tONE = cw.tile([128, 1], dt.float32, tag="one", name="one")
            nc.vector.memset(tONE[:], 1.0)
            tRES = cw.tile([1, 4], dt.float32, tag="res", name="res")

            # ---------------- negative-chunk helpers ----------------
            tPARK = cw.tile([128, max(parkn, 1), HKC, SC], dt.float8e4,
                            tag="park", name="park")

            def l1_matmuls(m, dst4):
                """W1b @ negt chunk m into four 1-bank PSUM tiles."""
                for ht in range(HKC):
                    p1 = ps2.tile([128, 512], dt.float32, tag="ps")
                    for g in range(2):
                        nc.tensor.matmul(
                            p1[:, :SC],
                            lhsT=tW1B[:, g, :, ht * 128:(ht + 1) * 128],
                            rhs=tNEG[:, m, 2 * g:2 * g + 2, :],
                            start=(g == 0), stop=(g == 1), perf_mode=DR)
                    dst4[ht] = p1

            # ---- per-fi tail: DRAM bounces per piece (8/4/4 chunks); the
            # softplus block runs in two stages so only the tiny last piece
            # is serialized after the final chunk ----
            POFF = (0, NH // 2, 3 * NH // 4, NH)

            def bounce_piece(o, pc):
                c0, c1 = POFF[pc], POFF[pc + 1]
                nc.sync.dma_start(
                    out=tLV[:, (o * NH + c0) // 128:(o * NH + c1) // 128],
                    in_=dROW[o:o + 1, c0:c1]
                    .rearrange("a (p c) -> (a p) c", p=128))

            def bounce_pos(o):
                pcol = PH // 128
                nc.sync.dma_start(
                    out=tLPV[:, o * pcol:(o + 1) * pcol],
                    in_=dROW[o:o + 1, NH:]
                    .rearrange("a (p c) -> (a p) c", p=128))

            def tail_stage1():
                # everything except the last piece of the last fi; exps
                # grouped before lns so each act table loads exactly once
                w = (NSLOT - NH // 4) // 128
                nc.scalar.activation(out=tLV[:, :w], in_=tLV[:, :w],
                                     func=AF.Exp, bias=tB3C[:, 0:1])
                nc.scalar.activation(out=tLPV[:], in_=tLPV[:], func=AF.Exp,
                                     scale=-1.0, bias=tB3C[:, 1:2])
                nc.vector.tensor_mul(out=tLV[:, :w], in0=tLV[:, :w],
                                     in1=tMSKN[:, :w])
                nc.vector.tensor_mul(out=tLPV[:], in0=tLPV[:], in1=tMSKP[:])
                nc.scalar.activation(out=tLV[:, :w], in_=tLV[:, :w],
                                     func=AF.Ln, bias=1.0,
                                     accum_out=tAN[:, 1:2])
                nc.scalar.activation(out=tLPV[:], in_=tLPV[:], func=AF.Ln,
                                     bias=1.0, accum_out=tAN[:, 0:1])

            def tail_stage2():
                # rows already hold exp(x+b3); only mul + ln remain and the
                # ln table is already loaded
                w = (NSLOT - NH // 4) // 128
                nc.vector.tensor_mul(out=tLV[:, w:], in0=tLV[:, w:],
                                     in1=tMSKN[:, w:])
                nc.scalar.activation(out=tLV[:, w:], in_=tLV[:, w:],
                                     func=AF.Ln, bias=1.0,
                                     accum_out=tAN[:, 2:3])
                nc.vector.tensor_add(out=tAN[:, 1:2], in0=tAN[:, 1:2],
                                     in1=tAN[:, 2:3])
                for col, srcc in ((0, 0), (1, 1)):
                    pr = ps2.tile([128, 512], dt.float32, tag="ps",
                                  name="pr")
                    nc.tensor.matmul(pr[:1, :1],
                                     lhsT=tAN[:, srcc:srcc + 1],
                                     rhs=tONE[:], start=True, stop=True)
                    nc.vector.tensor_copy(out=tRES[0:1, col:col + 1],
                                          in_=pr[:1, :1])
                nc.vector.memset(tRES[0:1, 2:4], 0.0)
                nc.sync.dma_start(out=out[:], in_=tRES[:])

            # ---------------- per-f section (generator) ----------------
            # fi below is the consumption-order position (0 = first ready);
            # chunk indices, tAT/ftt/mask columns all use this ordering.
            def emit_f_section(fi, n8):
                cols = slice(fi * P, (fi + 1) * P)
                # AT = W1a @ fp + b1  (fp = n8) — MUST run at its step (n8
                # is recycled two steps later)
                for ht in range(HKC):
                    p1 = ps2.tile([128, 512], dt.float32, tag="ps")
                    for g in range(2):
                        nc.tensor.matmul(
                            p1[:, :P],
                            lhsT=tW1A[:, g, :, ht * 128:(ht + 1) * 128],
                            rhs=n8[:, 2 * g:2 * g + 2, :],
                            start=(g == 0), stop=(g == 1), perf_mode=DR)
                    nc.scalar.activation(
                        out=tAT[:, ht, cols], in_=p1[:, :P],
                        func=AF.Identity, bias=tB1[:, ht:ht + 1])
                yield
                # positives: h1 = relu(W1b@ft + AT); h2 = relu(W2@h1+b2)
                h1 = ng.tile([128, HKC, P], dt.float8e4, tag="h1", name="h1p")
                for ht in range(HKC):
                    p1 = ps2.tile([128, 512], dt.float32, tag="ps")
                    for g in range(2):
                        nc.tensor.matmul(
                            p1[:, :P],
                            lhsT=tW1B[:, g, :, ht * 128:(ht + 1) * 128],
                            rhs=tFTT[:, 2 * g:2 * g + 2, cols],
                            start=(g == 0), stop=(g == 1), perf_mode=DR)
                    nc.vector._custom_dve(
                        RELU_ADD, out=h1[:, ht, :], in0=p1[:, :P],
                        in1=tAT[:, ht, cols])
                yield
                h2 = ng.tile([128, HKC, P], dt.float8e4, tag="h2", name="h2p")
                for ht in range(HKC):
                    p1 = ps2.tile([128, 512], dt.float32, tag="ps")
                    for g in range(2):
                        nc.tensor.matmul(
                            p1[:, :P],
                            lhsT=tW2[:, g, :, ht * 128:(ht + 1) * 128],
                            rhs=h1[:, 2 * g:2 * g + 2, :],
                            start=(g == 0), stop=(g == 1), perf_mode=DR)
                    nc.scalar.activation(
                        out=h2[:, ht, :], in_=p1[:, :P],
                        func=AF.Relu, bias=tB2[:, ht:ht + 1])
                pl = ps2.tile([128, 512], dt.float32, tag="ps")
                for g in range(2):
                    nc.tensor.matmul(
                        pl[:, :P], lhsT=tW3[:, g],
                        rhs=h2[:, 2 * g:2 * g + 2, :],
                        start=(g == 0), stop=(g == 1), perf_mode=DR)
                row = lr.tile([1, SC], dt.bfloat16, tag="lrow", name="rowp")
                nc.scalar.activation(out=row[0:1, :P], in_=pl[0:1, :P],
                                     func=AF.Identity)
                nc.sync.dma_start(out=dROW[fi:fi + 1, NH:],
                                  in_=row[0:1, :P])
                bounce_pos(fi)
                yield
                # negatives, software-pipelined: L1(m) | L2(m-1) | L3(m-2)
                h1s, h2s = {}, {}
                for m in range(SCF + 2):
                    if m < SCF:
                        cm = fi * SCF + m
                        a0 = fi * P + m * 24
                        h1n = ng.tile([128, HKC, SC], dt.float8e4, tag="h1",
                                      name="h1n")
                        if cm < parkn:
                            for ht in range(HKC):
                                nc.vector._custom_dve(
                                    RELU_ADD,
                                    out=h1n[:, ht, :].rearrange(
                                        "p (a b) -> p a b", b=NNEG),
                                    in0=tPARK[:, cm, ht, :].rearrange(
                                        "p (a b) -> p a b", b=NNEG),
                                    in1=tAT[:, ht, a0:a0 + 24][:, :, None]
                                    .broadcast_to((128, 24, NNEG)))
                        else:
                            p4 = [None] * HKC
                            l1_matmuls(cm, p4)
                            for ht in range(HKC):
                                nc.vector._custom_dve(
                                    RELU_ADD,
                                    out=h1n[:, ht, :].rearrange(
                                        "p (a b) -> p a b", b=NNEG),
                                    in0=p4[ht][:, :SC].rearrange(
                                        "p (a b) -> p a b", b=NNEG),
                                    in1=tAT[:, ht, a0:a0 + 24][:, :, None]
                                    .broadcast_to((128, 24, NNEG)))
                        h1s[m] = h1n
                    if 1 <= m <= SCF:
                        h1n = h1s.pop(m - 1)
                        h2n = ng.tile([128, HKC, SC], dt.float8e4, tag="h2",
                                      name="h2n")
                        for ht in range(HKC):
                            p1 = ps2.tile([128, 512], dt.float32, tag="ps")
                            for g in range(2):
                                nc.tensor.matmul(
                                    p1[:, :SC],
                                    lhsT=tW2[:, g, :,
                                             ht * 128:(ht + 1) * 128],
                                    rhs=h1n[:, 2 * g:2 * g + 2, :],
                                    start=(g == 0), stop=(g == 1),
                                    perf_mode=DR)
                            nc.scalar.activation(
                                out=h2n[:, ht, :], in_=p1[:, :SC],
                                func=AF.Relu, bias=tB2[:, ht:ht + 1])
                        h2s[m - 1] = h2n
                    if m >= 2:
                        ml = m - 2
                        h2n = h2s.pop(ml)
                        pl = ps2.tile([128, 512], dt.float32, tag="ps")
                        for g in range(2):
                            nc.tensor.matmul(
                                pl[:, :SC], lhsT=tW3[:, g],
                                rhs=h2n[:, 2 * g:2 * g + 2, :],
                                start=(g == 0), stop=(g == 1), perf_mode=DR)
                        row = lr.tile([1, SC], dt.bfloat16, tag="lrow",
                                      name="rown")
                        if fi == FS - 1 and ml >= 3 * SCF // 4:
                            # last piece: store exp(x+b3) so the final stage
                            # needs only the ln table (no extra table swap)
                            nc.scalar.activation(out=row[:], in_=pl[0:1, :SC],
                                                 func=AF.Exp,
                                                 bias=tB3C[0:1, 0:1])
                        elif ml % 2 == 0:
                            nc.vector.tensor_copy(out=row[:], in_=pl[0:1, :SC])
                        else:
                            nc.scalar.activation(out=row[:], in_=pl[0:1, :SC],
                                                 func=AF.Identity)
                        nc.sync.dma_start(
                            out=dROW[fi:fi + 1, ml * SC:(ml + 1) * SC],
                            in_=row[:])
                    if m - 2 == SCF // 2 - 1:
                        bounce_piece(fi, 0)
                    elif m - 2 == 3 * SCF // 4 - 1:
                        bounce_piece(fi, 1)
                    if fi == FS - 1 and m == SCF + 1:
                        # Scalar is past its last relu; exp/ln are grouped so
                        # each table loads once, hidden inside the drain
                        tail_stage1()
                    yield
                bounce_piece(fi, 2)
                if fi == FS - 1:
                    tail_stage2()

            def park_chunk(m):
                """L1 matmuls for chunk m + pre-activation parked to SBUF
                fp8 (copies split DVE/Scalar; AT-add applied post-scan).
                Bridges the per-step chain-tail PE gap so the PE pstate
                stays ramped."""
                p4 = [None] * HKC
                l1_matmuls(m, p4)
                for ht in range(HKC):
                    eng = nc.vector.tensor_copy if ht % 2 == 0 else None
                    if eng:
                        eng(out=tPARK[:, m, ht, :], in_=p4[ht][:, :SC])
                    else:
                        nc.scalar.activation(out=tPARK[:, m, ht, :],
                                             in_=p4[ht][:, :SC],
                                             func=AF.Identity)

            # ---------------- GRU scan + interleaving ----------------
            parked = [0]
            pending = []
            for k in range(K):
                c8, n8 = tC8[k % 2], tC8[(k + 1) % 2]

                # Each matmul group is split: kt01+aut passes depend only on
                # the previous step's cp0 state half, kt23 on cp1.  Emitting
                # all cp0-dependent passes first lets the PE bridge the
                # previous step's chain tail.
                def open_grp(gts, tag):
                    ps = []
                    for gt in gts:
                        p1 = ps2.tile([128, 512], dt.float32, tag="ps")
                        nc.tensor.matmul(
                            p1[:, :P],
                            lhsT=tWHH[:, 0, :, gt * 128:(gt + 1) * 128],
                            rhs=c8[:, 0:2, :],
                            start=True, stop=False, perf_mode=DR)
                        nc.tensor.matmul(
                            p1[:, :P],
                            lhsT=tWHH[:, 2, :, gt * 128:(gt + 1) * 128],
                            rhs=tAUT[:, k],
                            start=False, stop=False, perf_mode=DR)
                        ps.append(p1)
                    return ps

                def close_grp(ps, gts):
                    for p1, gt in zip(ps, gts):
                        nc.tensor.matmul(
                            p1[:, :P],
                            lhsT=tWHH[:, 1, :, gt * 128:(gt + 1) * 128],
                            rhs=c8[:, 2:4, :],
                            start=False, stop=True, perf_mode=DR)

                def sig(ps, dst, cp, split=False):
                    for j in range(2):
                        nc.scalar.activation(
                            out=dst[:, cp + j, :], in_=ps[j][:, :P],
                            func=AF.Sigmoid)

                def chain_a(cp, phs):
                    # per-j (single ct tile) ops halve each link's latency
                    # on the serial path; j1 trails j0 by one engine slot
                    t2 = ng.tile([128, 2, P], dt.bfloat16, tag="tm", name="t2")
                    c2 = ng.tile([128, 2, P], dt.bfloat16, tag="tm", name="c2")
                    d2 = ng.tile([128, 2, P], dt.bfloat16, tag="tm", name="d2")
                    for j in range(2):
                        ct = cp * 2 + j
                        # psum = gh_n + bhh_n (bias row in the aut pass), so
                        # the r-mult is a plain tensor_tensor
                        nc.vector.tensor_mul(out=t2[:, j, :],
                                             in0=phs[j][:, :P],
                                             in1=tR[:, ct, :])
                        nc.vector.tensor_add(out=t2[:, j, :], in0=t2[:, j, :],
                                             in1=tGIN[:, k, ct, :])
                        nc.scalar.activation(out=c2[:, j, :], in_=t2[:, j, :],
                                             func=AF.Tanh)
                        nc.vector.tensor_sub(out=d2[:, j, :],
                                             in0=c8[:, ct, :],
                                             in1=c2[:, j, :])
                    return c2, d2

                def chain_b(cp, c2, d2):
                    for j in range(2):
                        ct = cp * 2 + j
                        nc.vector.tensor_mul(out=d2[:, j, :], in0=d2[:, j, :],
                                             in1=tZ[:, ct, :])
                        # fp8 state: the final add writes the next-step state
                        # tile directly (no separate bf16 state / cast)
                        nc.vector.tensor_add(out=n8[:, ct, :],
                                             in0=d2[:, j, :],
                                             in1=c2[:, j, :])

                if k >= 3 and parked[0] < parkn:
                    park_chunk(parked[0]); parked[0] += 1
                # phase 1: cp0-dependent partials for r01, r23, n0
                pA = open_grp((0, 1), "r01")
                pB = open_grp((2, 3), "r23")
                pC = open_grp((8, 9), "n0")
                # phase 2: cp1-dependent closes; n0 closes right after the
                # r01 sigmoid so the cp0 chain launches ASAP
                close_grp(pA, (0, 1))
                sig(pA, tR, 0, split=True)
                close_grp(pC, (8, 9))
                ca0 = chain_a(0, pC)
                close_grp(pB, (2, 3))
                sig(pB, tR, 2, split=True)
                # n1 full group
                pD = []
                for gt in (10, 11):
                    p1 = ps2.tile([128, 512], dt.float32, tag="ps")
                    nc.tensor.matmul(
                        p1[:, :P],
                        lhsT=tWHH[:, 0, :, gt * 128:(gt + 1) * 128],
                        rhs=c8[:, 0:2, :],
                        start=True, stop=False, perf_mode=DR)
                    nc.tensor.matmul(
                        p1[:, :P],
                        lhsT=tWHH[:, 2, :, gt * 128:(gt + 1) * 128],
                        rhs=tAUT[:, k],
                        start=False, stop=False, perf_mode=DR)
                    nc.tensor.matmul(
                        p1[:, :P],
                        lhsT=tWHH[:, 1, :, gt * 128:(gt + 1) * 128],
                        rhs=c8[:, 2:4, :],
                        start=False, stop=True, perf_mode=DR)
                    pD.append(p1)
                pE = open_grp((4, 5), "z01")
                close_grp(pE, (4, 5))
                sig(pE, tZ, 0)
                chain_b(0, *ca0)
                ca1 = chain_a(1, pD)
                pF = open_grp((6, 7), "z23")
                close_grp(pF, (6, 7))
                sig(pF, tZ, 2)
                chain_b(1, *ca1)
                                # chain-tail filler: parked L1 chunks keep the PE busy
                # (and its pstate ramped) across the serial chain tail
                if parked[0] + 2 <= parkn:
                    park_chunk(parked[0])
                    park_chunk(parked[0] + 1)
                    parked[0] += 2
                for fi in range(FS):
                    if usub_vals[forder[fi]] == k:
                        sec = emit_f_section(fi, n8)
                        next(sec)           # AT stage now; rest post-scan
                        pending.append(sec)
                if k == K - 1:
                    while pending:
                        try:
                            next(pending[0])
                        except StopIteration:
                            pending.pop(0)

    nc.compile()
    return nc


def _get_program(usub_vals):
    key = usub_vals
    if key not in _PROG_CACHE:
        _PROG_CACHE[key] = _build(usub_vals)
    return _PROG_CACHE[key]


def kernel(**inputs):
    from concourse.bass_utils import run_bass_kernel_spmd
    in_maps, usub_vals, denom = _prep(inputs)
    nc = _get_program(usub_vals)
    res = run_bass_kernel_spmd(nc, in_maps, list(range(NCORE)))
    parts = np.stack([np.asarray(res.results[c]['out'][0], np.float64)
                      for c in range(NCORE)])
    pos, neg = parts[:, 0].sum(), parts[:, 1].sum()
    return np.float32(0.1 * (pos / denom + neg / (denom * NNEG)))
